# revision 40
# baseline (speedup 1.0000x reference)
"""Two-layer GAT on 8 Trainium2 NeuronCores (Bass/Tile, no collectives).

v2 design
---------
* 3 device launches, host does only data marshaling between them:
  - T1: data-parallel transform x@W1p -> table1 shard (each core 1/8 of
    nodes) + per-core max of the a_src scores (softmax bound).
  - E1: edge pass layer 1 (gather + segment softmax + weighted sum) with
    the FULL table1 as a pre-staged input; fuses the layer-2 transform
    (elu(h)@W2p) so its output is directly the table2 shard.
  - E2: edge pass layer 2 -> final [N, 40] rows.
* Node ids relabeled into G=4 groups of GSZ rows (int16 gather indices,
  idx local to group). One pad row per group (a_src = -60000 => exp -> 0).
* Destination nodes are assigned to (core, tile) slots via a Morton-order
  sort of their per-group in-degree profiles: all 8 cores share the same
  per-tile slot counts D_tbl[t,g] (one program for all cores), and Morton
  clustering minimizes sum(max) padding (~1.45x vs 2.13x for degree sort).
* Edge phase per 128-dst tile: dma_gather all incoming rows (one call per
  group, 256B rows = [h | ones | a_s | a_d]), unnormalized-softmax with a
  per-(dst,head) upper bound m' = leaky(C + a_d) so exp stays in fp16,
  denominator via "ones" columns of the same multiply, message sum via
  in-place halving tree adds (packed fp16 -> 2x/4x DVE modes).
"""
import sys
sys.path.insert(0, "/opt/trn_rl_repo")

import numpy as np

P = 128
NEG = -60000.0

_NC_CACHE = {}


def _mybir():
    from concourse import mybir
    return mybir


def make_cfg(n_raw=100000, f_in=512, hh=8, cc=8, out_w=40, ncores=8, gsz=25088):
    g = 4
    sh_pad = 12544                       # 98 tiles * 128 rows per core
    nt = sh_pad // P
    assert ncores * sh_pad == g * gsz    # shard rows == table rows
    return dict(
        N_RAW=n_raw, F_IN=f_in, HH=hh, CC=cc, F_HID=hh * cc, OUT_W=out_w,
        NCORES=ncores, NTILES=nt, SHPAD=sh_pad,
        G=g, GSZ=gsz, PAD_LOCAL=gsz - 1, NV=g * gsz,
    )


# --------------------------------------------------------------------------
# host-side graph prep
# --------------------------------------------------------------------------

def _morton_key(c):
    m = np.zeros(len(c), dtype=np.int64)
    for b in range(6):
        for g in range(c.shape[1]):
            m |= ((c[:, g].astype(np.int64) >> b) & 1) << (b * c.shape[1] + g)
    return m


def prep_graph(cfg, src_orig, dst_orig):
    """Morton-window node placement + per-core gather-index blobs.

    Returns (D_tbl [NT,G], blobs [ncores x [128, CB] int16],
             orows [ncores x [SHPAD] original ids, -1 = empty], tile_off, CB)
    """
    G, GSZ = cfg["G"], cfg["GSZ"]
    NT, SHPAD, NC = cfg["NTILES"], cfg["SHPAD"], cfg["NCORES"]
    N = cfg["N_RAW"]
    PAD_LOCAL = cfg["PAD_LOCAL"]

    src = np.asarray(src_orig, dtype=np.int64)
    dst = np.asarray(dst_orig, dtype=np.int64)

    # per-node per-group edge counts (self-loop handled via direct DMA slot)
    c = np.zeros((N, G), dtype=np.int32)
    np.add.at(c, (dst, src % G), 1)
    own = (np.arange(N) % G).astype(np.int64)

    order = np.argsort(_morton_key(c), kind="stable")
    W = NC * P
    ordp = np.full(NT * W, -1, dtype=np.int64)
    ordp[:N] = order
    owin = ordp.reshape(NT, NC, P)
    orows = [owin[:, r, :].ravel().copy() for r in range(NC)]

    core_of = np.full(N, -1, dtype=np.int64)
    pos_of = np.full(N, -1, dtype=np.int64)
    for r in range(NC):
        v = orows[r]
        m = v >= 0
        core_of[v[m]] = r
        pos_of[v[m]] = np.nonzero(m)[0]

    cnts = []
    per_core_fill = []
    for r in range(NC):
        em = core_of[dst] == r
        es = src[em]
        ep = pos_of[dst[em]]
        sg = es % G
        sl = (es // G).astype(np.int16)
        key = ep * G + sg
        o = np.argsort(key, kind="stable")
        ks, vs = key[o], sl[o]
        cnt = np.bincount(ks, minlength=SHPAD * G).reshape(SHPAD, G)
        starts = np.zeros(SHPAD * G, dtype=np.int64)
        np.cumsum(cnt.ravel()[:-1], out=starts[1:])
        col = np.arange(len(ks)) - starts[ks]
        cnts.append(cnt)
        per_core_fill.append((ks, vs, col))

    cnt_all = np.stack(cnts)                          # [NC, SHPAD, G]
    D_tbl = cnt_all.reshape(NC, NT, P, G).max(axis=(0, 2))      # [NT, G]
    D_tbl = np.maximum(D_tbl, 1)
    dmax = int(D_tbl.max())

    tile_off = []
    off = 0
    for t in range(NT):
        tile_off.append(off)
        off += 8 * int(D_tbl[t].sum())
    CB = off

    blobs = []
    for r in range(NC):
        ks, vs, col = per_core_fill[r]
        big = np.full((SHPAD, G, dmax), PAD_LOCAL, dtype=np.int16)
        big[ks // G, ks % G, col] = vs

        blob = np.empty((P, CB), dtype=np.int16)
        for t in range(NT):
            cpos = tile_off[t]
            for g in range(G):
                D = int(D_tbl[t, g])
                mat = big[t * P:(t + 1) * P, g, :D]      # [128, D]
                L = mat.T.ravel()                         # i = j*128 + p
                W16 = L.reshape(-1, 16).T                 # [16, 8*D]
                blob[:, cpos:cpos + 8 * D] = np.tile(W16, (8, 1))
                cpos += 8 * D
        blobs.append(blob)

    return D_tbl, blobs, orows, tile_off, CB


# --------------------------------------------------------------------------
# device programs
# --------------------------------------------------------------------------

def build_transform(cfg):
    """T1: fT shard [512, SHPAD] @ Wp [512, 96] -> tshard [SHPAD, 96] fp16
    (row = [h64 | ones8 | as8 | ad8 | 0*8]), plus column-max of as -> cmo."""
    import concourse.bass as bass
    import concourse.bacc as bacc
    import concourse.tile as tile
    mybir = _mybir()
    f16, f32 = mybir.dt.float16, mybir.dt.float32

    F_IN, SHPAD, NT = cfg["F_IN"], cfg["SHPAD"], cfg["NTILES"]
    RW = 96

    nc = bacc.Bacc("TRN2", target_bir_lowering=False, debug=False)
    fT = nc.dram_tensor("fT", [F_IN, SHPAD], f16, kind="ExternalInput")
    Wp = nc.dram_tensor("Wp", [F_IN, RW], f16, kind="ExternalInput")
    tshard = nc.dram_tensor("tshard", [SHPAD, RW], f16, kind="ExternalOutput")

    KCH = [(k, min(P, F_IN - k)) for k in range(0, F_IN, P)]

    with tile.TileContext(nc) as tc:
        import contextlib
        with contextlib.ExitStack() as ctx:
            singles = ctx.enter_context(tc.tile_pool(name="singles", bufs=1))
            xtp = ctx.enter_context(tc.tile_pool(name="xt", bufs=3))
            psp = ctx.enter_context(tc.tile_pool(name="ps", bufs=4, space="PSUM"))
            otp = ctx.enter_context(tc.tile_pool(name="ot", bufs=3))
            cp = ctx.enter_context(tc.tile_pool(name="cp", bufs=2))

            wts = []
            for kc, (k0, kn) in enumerate(KCH):
                wt = singles.tile([P, RW], f16, tag=f"w{kc}")
                nc.sync.dma_start(out=wt[:kn, :], in_=Wp[k0:k0 + kn, :])
                wts.append(wt)
            onesrow = singles.tile([P, RW], f32)
            nc.vector.memset(onesrow[:], 0.0)
            nc.vector.memset(onesrow[:, 64:72], 1.0)

            NKC = len(KCH)
            for t in range(NT):
                xt4 = xtp.tile([P, NKC, P], f16, tag="x4")
                nc.sync.dma_start(
                    out=xt4[:],
                    in_=fT[:, t * P:(t + 1) * P]
                        .rearrange("(c k) n -> k c n", k=P))
                pt = psp.tile([P, RW], f32)
                for kc in range(NKC):
                    nc.tensor.matmul(out=pt[:], lhsT=xt4[:, kc, :],
                                     rhs=wts[kc][:],
                                     start=(kc == 0), stop=(kc == NKC - 1))
                ot = otp.tile([P, RW], f16)
                nc.vector.tensor_add(out=ot[:], in0=pt[:], in1=onesrow[:])
                nc.sync.dma_start(out=tshard[t * P:(t + 1) * P, :], in_=ot[:])

    nc.compile()
    return nc


def build_edge(cfg, layer, D_tbl, tile_off, CB):
    """Edge pass. layer 1: row [h64|ones8|as8|ad8], fused table2 emit.
    layer 2: row [g40|ones1|as1|ad1], final output rows."""
    import concourse.bass as bass
    import concourse.bacc as bacc
    import concourse.tile as tile
    mybir = _mybir()
    f16, f32, i16 = mybir.dt.float16, mybir.dt.float32, mybir.dt.int16

    G, GSZ, NV, NT = cfg["G"], cfg["GSZ"], cfg["NV"], cfg["NTILES"]
    SHPAD = cfg["SHPAD"]
    HH = cfg["HH"] if layer == 1 else 1
    CC = cfg["CC"] if layer == 1 else cfg["OUT_W"]
    CT = HH * CC
    MB = CT + HH                        # msg block incl ones cols
    AS_OFF, AD_OFF = MB, MB + HH
    OUT_W = CT
    fused = (layer == 1)
    W2RW = 48

    nc = bacc.Bacc("TRN2", target_bir_lowering=False, debug=False,
                   num_swdge_queues=4)
    table = nc.dram_tensor("table", [NV, 128], f16, kind="ExternalInput")
    ownr = nc.dram_tensor("ownr", [SHPAD, 128], f16, kind="ExternalInput")
    idxb = nc.dram_tensor("idxb", [P, CB], i16, kind="ExternalInput")
    bias = nc.dram_tensor("bias", [OUT_W], f32, kind="ExternalInput")
    cbnd = nc.dram_tensor("cbnd", [HH], f32, kind="ExternalInput")
    if fused:
        w2p = nc.dram_tensor("w2p", [CT, W2RW], f16, kind="ExternalInput")
        ident = nc.dram_tensor("ident", [P, P], f16, kind="ExternalInput")
        t2o = nc.dram_tensor("t2o", [SHPAD, W2RW], f16, kind="ExternalOutput")
    else:
        outo = nc.dram_tensor("out", [SHPAD, OUT_W], f32, kind="ExternalOutput")

    qn = [0]

    def nextq():
        q = qn[0]
        qn[0] = (qn[0] + 1) % 4
        return q

    with tile.TileContext(nc) as tc:
        import contextlib
        with contextlib.ExitStack() as ctx:
            singles = ctx.enter_context(tc.tile_pool(name="singles", bufs=1))
            ip = ctx.enter_context(tc.tile_pool(name="ip", bufs=4))
            gp = ctx.enter_context(tc.tile_pool(name="gp", bufs=5))
            ep = ctx.enter_context(tc.tile_pool(name="ep", bufs=3))
            xp = ctx.enter_context(tc.tile_pool(name="xp", bufs=2))
            mp = ctx.enter_context(tc.tile_pool(name="mp", bufs=2))
            sp = ctx.enter_context(tc.tile_pool(name="sp", bufs=4))
            otp = ctx.enter_context(tc.tile_pool(name="otp", bufs=3))
            cp = ctx.enter_context(tc.tile_pool(name="cp", bufs=2))
            if fused:
                pstp = ctx.enter_context(
                    tc.tile_pool(name="pst", bufs=2, space="PSUM"))
                ps2p = ctx.enter_context(
                    tc.tile_pool(name="ps2", bufs=2, space="PSUM"))

            # ---- singles ----
            bias_ap = bias[:]
            bias_b = bass.AP(tensor=bias_ap.tensor, offset=bias_ap.offset,
                             ap=[[0, P]] + list(bias_ap.ap))
            bt = singles.tile([P, OUT_W], f32)
            nc.sync.dma_start(out=bt[:], in_=bias_b)
            cb_ap = cbnd[:]
            cb_b = bass.AP(tensor=cb_ap.tensor, offset=cb_ap.offset,
                           ap=[[0, P]] + list(cb_ap.ap))
            cbf = singles.tile([P, HH], f32)
            nc.sync.dma_start(out=cbf[:], in_=cb_b)
            cb16 = singles.tile([P, HH], f16)
            nc.vector.tensor_copy(out=cb16[:], in_=cbf[:])
            if fused:
                w2t = singles.tile([CT, W2RW], f16)
                nc.sync.dma_start(out=w2t[:], in_=w2p[:, :])
                idt = singles.tile([P, P], f16)
                nc.sync.dma_start(out=idt[:], in_=ident[:, :])
                ones2 = singles.tile([P, W2RW], f32)
                nc.vector.memset(ones2[:], 0.0)
                nc.vector.memset(ones2[:, 40:41], 1.0)

            for t in range(NT):
                Ds = [int(D_tbl[t, g]) for g in range(G)]
                SD = sum(Ds)
                SDP = SD + 1      # + own-row slot (direct DMA, no gather)
                it = ip.tile([P, 8 * SD], i16, tag="idx")
                nc.sync.dma_start(
                    out=it[:], in_=idxb[:, tile_off[t]:tile_off[t] + 8 * SD])

                Gt = gp.tile([P, SDP, 128], f16, tag="G")
                c0 = 0
                ic = 0
                GCH = 8           # 1024 descriptors per call (ucode ring limit)
                for g in range(G):
                    D = Ds[g]
                    for d0 in range(0, D, GCH):
                        dn = min(GCH, D - d0)
                        nc.gpsimd.dma_gather(
                            out_ap=Gt[:, c0:c0 + dn, :],
                            in_ap=table[g * GSZ:, :],
                            idxs_ap=it[:, ic:ic + 8 * dn],
                            num_idxs=P * dn,
                            num_idxs_reg=P * dn,
                            elem_size=128,
                            queue_num=nextq(),
                        )
                        c0 += dn
                        ic += 8 * dn
                nc.sync.dma_start(out=Gt[:, SD, :],
                                  in_=ownr[t * P:(t + 1) * P, :])

                # ad_own [p, H] straight from the own-row slot
                adt = sp.tile([P, HH], f16, tag="ad")
                nc.vector.tensor_copy(out=adt[:],
                                      in_=Gt[:, SD, AD_OFF:AD_OFF + HH])

                # nm = -leaky(C + ad_own)   [p, H] f16
                nm = sp.tile([P, HH], f16, tag="nm")
                nc.vector.tensor_add(out=nm[:], in0=adt[:], in1=cb16[:])
                nc.vector.scalar_tensor_tensor(
                    out=nm[:], in0=nm[:], scalar=0.2, in1=nm[:],
                    op0=mybir.AluOpType.mult, op1=mybir.AluOpType.max)
                nc.vector.tensor_scalar_mul(out=nm[:], in0=nm[:], scalar1=-1.0)

                # z = as + ad_own ; L = leaky(z) ; y = L - m' ; ex = exp(y)
                zt = ep.tile([P, SDP, HH], f16, tag="z")
                nc.vector.tensor_tensor(
                    out=zt[:], in0=Gt[:, :, AS_OFF:AS_OFF + HH],
                    in1=adt[:].unsqueeze(1).broadcast_to([P, SDP, HH]),
                    op=mybir.AluOpType.add)
                zf = zt[:].rearrange("p d h -> p (d h)")
                nc.vector.scalar_tensor_tensor(
                    out=zf, in0=zf, scalar=0.2, in1=zf,
                    op0=mybir.AluOpType.mult, op1=mybir.AluOpType.max)
                nc.vector.tensor_tensor(
                    out=zt[:], in0=zt[:],
                    in1=nm[:].unsqueeze(1).broadcast_to([P, SDP, HH]),
                    op=mybir.AluOpType.add)
                ext = ep.tile([P, SDP, HH], f16, tag="ex")
                nc.scalar.activation(
                    out=ext[:].rearrange("p d h -> p (d h)"), in_=zf,
                    func=mybir.ActivationFunctionType.Exp)

                # expand ex -> [p, d, MB] (Act engine)
                exm = xp.tile([P, SDP, MB], f16, tag="exm")
                nc.scalar.activation(
                    out=exm[:, :, 0:CT].rearrange("p d (h c) -> p d h c", h=HH),
                    in_=ext[:].unsqueeze(3).broadcast_to([P, SDP, HH, CC]),
                    func=mybir.ActivationFunctionType.Copy)
                nc.scalar.activation(
                    out=exm[:, :, CT:MB], in_=ext[:],
                    func=mybir.ActivationFunctionType.Copy)

                # msg = feat * ex ; tree-reduce over slots
                mg = mp.tile([P, SDP, MB], f16, tag="mg")
                nc.vector.tensor_tensor(out=mg[:], in0=Gt[:, :, 0:MB],
                                        in1=exm[:], op=mybir.AluOpType.mult)
                cur = SDP
                while cur > 2:
                    half = cur // 2
                    nc.vector.tensor_add(out=mg[:, 0:half],
                                         in0=mg[:, 0:half],
                                         in1=mg[:, cur - half:cur])
                    cur = cur - half
                redf = sp.tile([P, MB], f32, tag="red")
                if cur == 2:
                    nc.vector.tensor_add(out=redf[:], in0=mg[:, 0], in1=mg[:, 1])
                else:
                    nc.vector.tensor_copy(out=redf[:], in_=mg[:, 0])

                # normalize + bias
                rd = sp.tile([P, HH], f32, tag="rd")
                nc.vector.tensor_scalar_add(out=rd[:], in0=redf[:, CT:MB],
                                            scalar1=1e-16)
                nc.vector.reciprocal(out=rd[:], in_=rd[:])
                o1 = sp.tile([P, CT], f32, tag="o1")
                nc.vector.tensor_tensor(
                    out=o1[:].rearrange("p (h c) -> p h c", h=HH),
                    in0=redf[:, 0:CT].rearrange("p (h c) -> p h c", h=HH),
                    in1=rd[:].unsqueeze(2).broadcast_to([P, HH, CC]),
                    op=mybir.AluOpType.mult)
                nc.vector.tensor_add(out=o1[:], in0=o1[:], in1=bt[:])

                if fused:
                    # elu -> fp16
                    t1 = sp.tile([P, CT], f32, tag="t1")
                    nc.vector.tensor_scalar_min(out=t1[:], in0=o1[:], scalar1=0.0)
                    nc.scalar.activation(out=t1[:], in_=t1[:],
                                         func=mybir.ActivationFunctionType.Exp)
                    t2e = sp.tile([P, CT], f32, tag="t2e")
                    nc.vector.tensor_scalar_max(out=t2e[:], in0=o1[:], scalar1=0.0)
                    nc.vector.tensor_add(out=t1[:], in0=t1[:], in1=t2e[:])
                    h16 = sp.tile([P, CT], f16, tag="h16")
                    nc.vector.tensor_scalar_add(out=h16[:], in0=t1[:], scalar1=-1.0)
                    # transpose h16 -> [CT, 128] then @ W2p -> table2 rows
                    psT = pstp.tile([CT, P], f16)
                    nc.tensor.transpose(out=psT[:], in_=h16[:], identity=idt[:])
                    hT = otp.tile([CT, P], f16, tag="hT")
                    nc.vector.tensor_copy(out=hT[:], in_=psT[:])
                    ps2 = ps2p.tile([P, W2RW], f32)
                    nc.tensor.matmul(out=ps2[:], lhsT=hT[:], rhs=w2t[:],
                                     start=True, stop=True)
                    t2t = otp.tile([P, W2RW], f16, tag="t2t")
                    nc.vector.tensor_add(out=t2t[:], in0=ps2[:], in1=ones2[:])
                    nc.sync.dma_start(out=t2o[t * P:(t + 1) * P, :], in_=t2t[:])
                else:
                    nc.sync.dma_start(out=outo[t * P:(t + 1) * P, :], in_=o1[:])

    nc.compile()
    return nc


# --------------------------------------------------------------------------
# host orchestration
# --------------------------------------------------------------------------

def _fold_w1(W1, a_src, a_dst, hh, cc):
    W1r = W1.reshape(W1.shape[0], hh, cc)
    ws = np.einsum("khc,hc->kh", W1r, a_src)
    wd = np.einsum("khc,hc->kh", W1r, a_dst)
    z8 = np.zeros((W1.shape[0], 8), np.float32)
    return np.concatenate([W1, z8, ws, wd, z8], axis=1).astype(np.float16)


def _fold_w2(W2, a_src2, a_dst2):
    z1 = np.zeros((W2.shape[0], 1), np.float32)
    z5 = np.zeros((W2.shape[0], 5), np.float32)
    return np.concatenate([W2, z1, (W2 @ a_src2[0])[:, None],
                           (W2 @ a_dst2[0])[:, None], z5], axis=1).astype(np.float16)


def _get_programs(cfg, D_tbl, tile_off, CB):
    key = (CB, D_tbl.tobytes())
    if key not in _NC_CACHE:
        _NC_CACHE[key] = dict(
            T1=build_transform(cfg),
            E1=build_edge(cfg, 1, D_tbl, tile_off, CB),
            E2=build_edge(cfg, 2, D_tbl, tile_off, CB),
        )
    return _NC_CACHE[key]


def _prep_all(x, edge_index, W1, a_src1, a_dst1, W2, a_src2, a_dst2):
    cfg = make_cfg()
    G, GSZ, NV, SHPAD = cfg["G"], cfg["GSZ"], cfg["NV"], cfg["SHPAD"]
    N = cfg["N_RAW"]
    src, dst = edge_index[0], edge_index[1]
    D_tbl, blobs, orows, tile_off, CB = prep_graph(cfg, src, dst)

    orig = np.arange(N, dtype=np.int64)
    new_id = (orig % G) * GSZ + orig // G

    fT_all = np.zeros((cfg["F_IN"], NV), dtype=np.float16)
    fT_all[:, new_id] = np.asarray(x).T.astype(np.float16)

    W1p = _fold_w1(np.asarray(W1), np.asarray(a_src1), np.asarray(a_dst1),
                   cfg["HH"], cfg["CC"])
    W2p = _fold_w2(np.asarray(W2), np.asarray(a_src2), np.asarray(a_dst2))
    ident = np.eye(P, dtype=np.float16)
    return dict(cfg=cfg, D_tbl=D_tbl, blobs=blobs, orows=orows,
                tile_off=tile_off, CB=CB, new_id=new_id, fT_all=fT_all,
                W1p=W1p, W2p=W2p, ident=ident)


def _run_spmd(nc, in_maps, ncores):
    from concourse.bass_utils import run_bass_kernel_spmd
    res = run_bass_kernel_spmd(nc, in_maps, list(range(ncores)))
    return res.results


def _t1_inmaps(pp):
    cfg = pp["cfg"]
    SHPAD = cfg["SHPAD"]
    return [{"fT": np.ascontiguousarray(pp["fT_all"][:, r * SHPAD:(r + 1) * SHPAD]),
             "Wp": pp["W1p"]} for r in range(cfg["NCORES"])]


def _assemble_table1(pp, outs):
    cfg = pp["cfg"]
    NV, G, GSZ = cfg["NV"], cfg["G"], cfg["GSZ"]
    table1 = np.zeros((NV, 128), dtype=np.float16)
    table1[:, 0:96] = np.vstack([o["tshard"] for o in outs])
    C1 = table1[:, 72:80].astype(np.float32).max(axis=0) + 0.02
    for g in range(G):
        table1[g * GSZ + cfg["PAD_LOCAL"], 72:80] = NEG
    return table1, np.ascontiguousarray(C1, dtype=np.float32)


def _own_rows(pp, table):
    cfg = pp["cfg"]
    rows = []
    for r in range(cfg["NCORES"]):
        v = pp["orows"][r]
        m = v >= 0
        arr = np.zeros((cfg["SHPAD"], 128), np.float16)
        arr[m] = table[pp["new_id"][v[m]]]
        rows.append(arr)
    return rows


def _e1_inmaps(pp, table1, C1, b1):
    cfg = pp["cfg"]
    own = _own_rows(pp, table1)
    return [{"table": table1, "ownr": own[r], "idxb": pp["blobs"][r],
             "bias": np.ascontiguousarray(b1, dtype=np.float32),
             "cbnd": C1, "w2p": pp["W2p"], "ident": pp["ident"]}
            for r in range(cfg["NCORES"])]


def _assemble_table2(pp, outs):
    cfg = pp["cfg"]
    NV, G, GSZ = cfg["NV"], cfg["G"], cfg["GSZ"]
    table2 = np.zeros((NV, 128), dtype=np.float16)
    for r in range(cfg["NCORES"]):
        v = pp["orows"][r]
        m = v >= 0
        table2[pp["new_id"][v[m]], 0:48] = outs[r]["t2o"][m]
    C2 = np.array([table2[:, 41].astype(np.float32).max() + 0.02],
                  dtype=np.float32)
    for g in range(G):
        table2[g * GSZ + cfg["PAD_LOCAL"], 41] = NEG
    return table2, C2


def _e2_inmaps(pp, table2, C2, b2):
    cfg = pp["cfg"]
    own = _own_rows(pp, table2)
    return [{"table": table2, "ownr": own[r], "idxb": pp["blobs"][r],
             "bias": np.ascontiguousarray(b2, dtype=np.float32),
             "cbnd": C2} for r in range(cfg["NCORES"])]


def _final_out(pp, outs):
    cfg = pp["cfg"]
    out = np.empty((cfg["N_RAW"], cfg["OUT_W"]), dtype=np.float32)
    for r in range(cfg["NCORES"]):
        v = pp["orows"][r]
        m = v >= 0
        out[v[m]] = outs[r]["out"][m]
    return out


def kernel(x, edge_index, W1, a_src1, a_dst1, b1, W2, a_src2, a_dst2, b2):
    x = np.asarray(x)
    edge_index = np.asarray(edge_index)
    pp = _prep_all(x, edge_index, W1, a_src1, a_dst1, W2, a_src2, a_dst2)
    cfg = pp["cfg"]
    progs = _get_programs(cfg, pp["D_tbl"], pp["tile_off"], pp["CB"])

    outs = _run_spmd(progs["T1"], _t1_inmaps(pp), cfg["NCORES"])
    table1, C1 = _assemble_table1(pp, outs)

    outs = _run_spmd(progs["E1"], _e1_inmaps(pp, table1, C1, np.asarray(b1)),
                     cfg["NCORES"])
    table2, C2 = _assemble_table2(pp, outs)

    outs = _run_spmd(progs["E2"], _e2_inmaps(pp, table2, C2, np.asarray(b2)),
                     cfg["NCORES"])
    return _final_out(pp, outs)


# --------------------------------------------------------------------------
# timing / profiling helpers (not used by the grader)
# --------------------------------------------------------------------------

def time_launches(inputs, repeats=2, hw=True):
    """Per-launch timing: TimelineSim (cost model) ns + optional HW walls.

    Returns {"T1": [sec...], "E1": [...], "E2": [...]} where values are
    sim exec times in seconds (so test.py's sum(min)*1e9 prints total ns).
    """
    import time as _time
    from concourse.timeline_sim import TimelineSim

    x = np.asarray(inputs["x"])
    ei = np.asarray(inputs["edge_index"])
    pp = _prep_all(x, ei, inputs["W1"], inputs["a_src1"], inputs["a_dst1"],
                   inputs["W2"], inputs["a_src2"], inputs["a_dst2"])
    cfg = pp["cfg"]
    progs = _get_programs(cfg, pp["D_tbl"], pp["tile_off"], pp["CB"])

    times = {}
    for name in ("T1", "E1", "E2"):
        tl = TimelineSim(progs[name], trace=False)
        ns = tl.simulate()
        times[name] = [ns / 1e9]
        print(f"  {name}: sim {ns:.0f} ns", flush=True)

    if hw:
        outs = _run_spmd(progs["T1"], _t1_inmaps(pp), cfg["NCORES"])
        table1, C1 = _assemble_table1(pp, outs)
        e1maps = _e1_inmaps(pp, table1, C1, np.asarray(inputs["b1"]))
        outs = _run_spmd(progs["E1"], e1maps, cfg["NCORES"])
        table2, C2 = _assemble_table2(pp, outs)
        e2maps = _e2_inmaps(pp, table2, C2, np.asarray(inputs["b2"]))
        for name, maps in (("T1", _t1_inmaps(pp)), ("E1", e1maps),
                           ("E2", e2maps)):
            walls = []
            for _ in range(repeats):
                t0 = _time.time()
                _run_spmd(progs[name], maps, cfg["NCORES"])
                walls.append(_time.time() - t0)
            print(f"  {name}: hw walls {[f'{w:.3f}' for w in walls]}", flush=True)
    return times


# revision 43
# speedup vs baseline: 1.0371x; 1.0371x over previous
"""Two-layer GAT on 8 Trainium2 NeuronCores (Bass/Tile, no collectives).

v2 design
---------
* 3 device launches, host does only data marshaling between them:
  - T1: data-parallel transform x@W1p -> table1 shard (each core 1/8 of
    nodes) + per-core max of the a_src scores (softmax bound).
  - E1: edge pass layer 1 (gather + segment softmax + weighted sum) with
    the FULL table1 as a pre-staged input; fuses the layer-2 transform
    (elu(h)@W2p) so its output is directly the table2 shard.
  - E2: edge pass layer 2 -> final [N, 40] rows.
* Node ids relabeled into G=4 groups of GSZ rows (int16 gather indices,
  idx local to group). One pad row per group (a_src = -60000 => exp -> 0).
* Destination nodes are assigned to (core, tile) slots via a Morton-order
  sort of their per-group in-degree profiles: all 8 cores share the same
  per-tile slot counts D_tbl[t,g] (one program for all cores), and Morton
  clustering minimizes sum(max) padding (~1.45x vs 2.13x for degree sort).
* Edge phase per 128-dst tile: dma_gather all incoming rows (one call per
  group, 256B rows = [h | ones | a_s | a_d]), unnormalized-softmax with a
  per-(dst,head) upper bound m' = leaky(C + a_d) so exp stays in fp16,
  denominator via "ones" columns of the same multiply, message sum via
  in-place halving tree adds (packed fp16 -> 2x/4x DVE modes).
"""
import sys
sys.path.insert(0, "/opt/trn_rl_repo")

import numpy as np

P = 128
NEG = -60000.0

_NC_CACHE = {}


def _mybir():
    from concourse import mybir
    return mybir


def make_cfg(n_raw=100000, f_in=512, hh=8, cc=8, out_w=40, ncores=8, gsz=25088):
    g = 4
    sh_pad = 12544                       # 98 tiles * 128 rows per core
    nt = sh_pad // P
    assert ncores * sh_pad == g * gsz    # shard rows == table rows
    return dict(
        N_RAW=n_raw, F_IN=f_in, HH=hh, CC=cc, F_HID=hh * cc, OUT_W=out_w,
        NCORES=ncores, NTILES=nt, SHPAD=sh_pad,
        G=g, GSZ=gsz, PAD_LOCAL=gsz - 1, NV=g * gsz,
    )


# --------------------------------------------------------------------------
# host-side graph prep
# --------------------------------------------------------------------------

def _morton_key(c):
    m = np.zeros(len(c), dtype=np.int64)
    for b in range(6):
        for g in range(c.shape[1]):
            m |= ((c[:, g].astype(np.int64) >> b) & 1) << (b * c.shape[1] + g)
    return m


def prep_graph(cfg, src_orig, dst_orig):
    """Morton-window node placement + per-core gather-index blobs.

    Returns (D_tbl [NT,G], blobs [ncores x [128, CB] int16],
             orows [ncores x [SHPAD] original ids, -1 = empty], tile_off, CB)
    """
    G, GSZ = cfg["G"], cfg["GSZ"]
    NT, SHPAD, NC = cfg["NTILES"], cfg["SHPAD"], cfg["NCORES"]
    N = cfg["N_RAW"]
    PAD_LOCAL = cfg["PAD_LOCAL"]

    src = np.asarray(src_orig, dtype=np.int64)
    dst = np.asarray(dst_orig, dtype=np.int64)

    # per-node per-group edge counts (self-loop handled via direct DMA slot)
    c = np.zeros((N, G), dtype=np.int32)
    np.add.at(c, (dst, src % G), 1)
    own = (np.arange(N) % G).astype(np.int64)

    order = np.argsort(_morton_key(c), kind="stable")
    W = NC * P
    ordp = np.full(NT * W, -1, dtype=np.int64)
    ordp[:N] = order
    owin = ordp.reshape(NT, NC, P)
    orows = [owin[:, r, :].ravel().copy() for r in range(NC)]

    core_of = np.full(N, -1, dtype=np.int64)
    pos_of = np.full(N, -1, dtype=np.int64)
    for r in range(NC):
        v = orows[r]
        m = v >= 0
        core_of[v[m]] = r
        pos_of[v[m]] = np.nonzero(m)[0]

    cnts = []
    per_core_fill = []
    for r in range(NC):
        em = core_of[dst] == r
        es = src[em]
        ep = pos_of[dst[em]]
        sg = es % G
        sl = (es // G).astype(np.int16)
        key = ep * G + sg
        o = np.argsort(key, kind="stable")
        ks, vs = key[o], sl[o]
        cnt = np.bincount(ks, minlength=SHPAD * G).reshape(SHPAD, G)
        starts = np.zeros(SHPAD * G, dtype=np.int64)
        np.cumsum(cnt.ravel()[:-1], out=starts[1:])
        col = np.arange(len(ks)) - starts[ks]
        cnts.append(cnt)
        per_core_fill.append((ks, vs, col))

    cnt_all = np.stack(cnts)                          # [NC, SHPAD, G]
    D_tbl = cnt_all.reshape(NC, NT, P, G).max(axis=(0, 2))      # [NT, G]
    D_tbl = np.maximum(D_tbl, 1)
    dmax = int(D_tbl.max())

    tile_off = []
    off = 0
    for t in range(NT):
        tile_off.append(off)
        off += 8 * int(D_tbl[t].sum())
    CB = off

    blobs = []
    for r in range(NC):
        ks, vs, col = per_core_fill[r]
        big = np.full((SHPAD, G, dmax), PAD_LOCAL, dtype=np.int16)
        big[ks // G, ks % G, col] = vs

        blob = np.empty((P, CB), dtype=np.int16)
        for t in range(NT):
            cpos = tile_off[t]
            for g in range(G):
                D = int(D_tbl[t, g])
                mat = big[t * P:(t + 1) * P, g, :D]      # [128, D]
                L = mat.T.ravel()                         # i = j*128 + p
                W16 = L.reshape(-1, 16).T                 # [16, 8*D]
                blob[:, cpos:cpos + 8 * D] = np.tile(W16, (8, 1))
                cpos += 8 * D
        blobs.append(blob)

    return D_tbl, blobs, orows, tile_off, CB


# --------------------------------------------------------------------------
# device programs
# --------------------------------------------------------------------------

def build_transform(cfg):
    """T1: fT shard [512, SHPAD] @ Wp [512, 96] -> tshard [SHPAD, 96] fp16
    (row = [h64 | ones8 | as8 | ad8 | 0*8]), plus column-max of as -> cmo."""
    import concourse.bass as bass
    import concourse.bacc as bacc
    import concourse.tile as tile
    mybir = _mybir()
    f16, f32 = mybir.dt.float16, mybir.dt.float32

    F_IN, SHPAD, NT = cfg["F_IN"], cfg["SHPAD"], cfg["NTILES"]
    RW = 96

    nc = bacc.Bacc("TRN2", target_bir_lowering=False, debug=False)
    fT = nc.dram_tensor("fT", [F_IN, SHPAD], f16, kind="ExternalInput")
    Wp = nc.dram_tensor("Wp", [F_IN, RW], f16, kind="ExternalInput")
    tshard = nc.dram_tensor("tshard", [SHPAD, RW], f16, kind="ExternalOutput")

    KCH = [(k, min(P, F_IN - k)) for k in range(0, F_IN, P)]

    with tile.TileContext(nc) as tc:
        import contextlib
        with contextlib.ExitStack() as ctx:
            singles = ctx.enter_context(tc.tile_pool(name="singles", bufs=1))
            xtp = ctx.enter_context(tc.tile_pool(name="xt", bufs=3))
            psp = ctx.enter_context(tc.tile_pool(name="ps", bufs=4, space="PSUM"))
            otp = ctx.enter_context(tc.tile_pool(name="ot", bufs=3))
            cp = ctx.enter_context(tc.tile_pool(name="cp", bufs=2))

            wts = []
            for kc, (k0, kn) in enumerate(KCH):
                wt = singles.tile([P, RW], f16, tag=f"w{kc}")
                nc.sync.dma_start(out=wt[:kn, :], in_=Wp[k0:k0 + kn, :])
                wts.append(wt)
            onesrow = singles.tile([P, RW], f32)
            nc.vector.memset(onesrow[:], 0.0)
            nc.vector.memset(onesrow[:, 64:72], 1.0)

            NKC = len(KCH)
            for t0 in range(0, NT, 2):
                nb = min(2, NT - t0)
                xt8 = xtp.tile([P, NKC, nb * P], f16, tag="x8")
                nc.sync.dma_start(
                    out=xt8[:],
                    in_=fT[:, t0 * P:(t0 + nb) * P]
                        .rearrange("(c k) n -> k c n", k=P))
                ot2 = otp.tile([P, nb, RW], f16, tag="ot2")
                for b in range(nb):
                    pt = psp.tile([P, RW], f32)
                    for kc in range(NKC):
                        nc.tensor.matmul(out=pt[:],
                                         lhsT=xt8[:, kc, b * P:(b + 1) * P],
                                         rhs=wts[kc][:],
                                         start=(kc == 0), stop=(kc == NKC - 1))
                    nc.vector.tensor_add(out=ot2[:, b], in0=pt[:], in1=onesrow[:])
                nc.sync.dma_start(
                    out=tshard[t0 * P:(t0 + nb) * P, :]
                        .rearrange("(b r) w -> r b w", r=P),
                    in_=ot2[:])

    nc.compile()
    return nc


def build_edge(cfg, layer, D_tbl, tile_off, CB):
    """Edge pass. layer 1: row [h64|ones8|as8|ad8], fused table2 emit.
    layer 2: row [g40|ones1|as1|ad1], final output rows."""
    import concourse.bass as bass
    import concourse.bacc as bacc
    import concourse.tile as tile
    mybir = _mybir()
    f16, f32, i16 = mybir.dt.float16, mybir.dt.float32, mybir.dt.int16

    G, GSZ, NV, NT = cfg["G"], cfg["GSZ"], cfg["NV"], cfg["NTILES"]
    SHPAD = cfg["SHPAD"]
    HH = cfg["HH"] if layer == 1 else 1
    CC = cfg["CC"] if layer == 1 else cfg["OUT_W"]
    CT = HH * CC
    MB = CT + HH                        # msg block incl ones cols
    AS_OFF, AD_OFF = MB, MB + HH
    OUT_W = CT
    fused = (layer == 1)
    W2RW = 48

    nc = bacc.Bacc("TRN2", target_bir_lowering=False, debug=False,
                   num_swdge_queues=4)
    table = nc.dram_tensor("table", [NV, 128], f16, kind="ExternalInput")
    ownr = nc.dram_tensor("ownr", [SHPAD, 128], f16, kind="ExternalInput")
    idxb = nc.dram_tensor("idxb", [P, CB], i16, kind="ExternalInput")
    bias = nc.dram_tensor("bias", [OUT_W], f32, kind="ExternalInput")
    cbnd = nc.dram_tensor("cbnd", [HH], f32, kind="ExternalInput")
    if fused:
        w2p = nc.dram_tensor("w2p", [CT, W2RW], f16, kind="ExternalInput")
        ident = nc.dram_tensor("ident", [P, P], f16, kind="ExternalInput")
        t2o = nc.dram_tensor("t2o", [SHPAD, W2RW], f16, kind="ExternalOutput")
    else:
        outo = nc.dram_tensor("out", [SHPAD, OUT_W], f32, kind="ExternalOutput")

    qn = [0]

    def nextq():
        q = qn[0]
        qn[0] = (qn[0] + 1) % 4
        return q

    with tile.TileContext(nc) as tc:
        import contextlib
        with contextlib.ExitStack() as ctx:
            singles = ctx.enter_context(tc.tile_pool(name="singles", bufs=1))
            ip = ctx.enter_context(tc.tile_pool(name="ip", bufs=4))
            gp = ctx.enter_context(tc.tile_pool(name="gp", bufs=5))
            ep = ctx.enter_context(tc.tile_pool(name="ep", bufs=3))
            xp = ctx.enter_context(tc.tile_pool(name="xp", bufs=2))
            mp = ctx.enter_context(tc.tile_pool(name="mp", bufs=2))
            sp = ctx.enter_context(tc.tile_pool(name="sp", bufs=4))
            otp = ctx.enter_context(tc.tile_pool(name="otp", bufs=3))
            cp = ctx.enter_context(tc.tile_pool(name="cp", bufs=2))
            if fused:
                pstp = ctx.enter_context(
                    tc.tile_pool(name="pst", bufs=2, space="PSUM"))
                ps2p = ctx.enter_context(
                    tc.tile_pool(name="ps2", bufs=2, space="PSUM"))

            # ---- singles ----
            bias_ap = bias[:]
            bias_b = bass.AP(tensor=bias_ap.tensor, offset=bias_ap.offset,
                             ap=[[0, P]] + list(bias_ap.ap))
            bt = singles.tile([P, OUT_W], f32)
            nc.sync.dma_start(out=bt[:], in_=bias_b)
            cb_ap = cbnd[:]
            cb_b = bass.AP(tensor=cb_ap.tensor, offset=cb_ap.offset,
                           ap=[[0, P]] + list(cb_ap.ap))
            cbf = singles.tile([P, HH], f32)
            nc.sync.dma_start(out=cbf[:], in_=cb_b)
            cb16 = singles.tile([P, HH], f16)
            nc.vector.tensor_copy(out=cb16[:], in_=cbf[:])
            if fused:
                w2t = singles.tile([CT, W2RW], f16)
                nc.sync.dma_start(out=w2t[:], in_=w2p[:, :])
                idt = singles.tile([P, P], f16)
                nc.sync.dma_start(out=idt[:], in_=ident[:, :])
                ones2 = singles.tile([P, W2RW], f32)
                nc.vector.memset(ones2[:], 0.0)
                nc.vector.memset(ones2[:, 40:41], 1.0)

            for t in range(NT):
                Ds = [int(D_tbl[t, g]) for g in range(G)]
                SD = sum(Ds)
                SDP = SD + 1      # + own-row slot (direct DMA, no gather)
                it = ip.tile([P, 8 * SD], i16, tag="idx")
                nc.sync.dma_start(
                    out=it[:], in_=idxb[:, tile_off[t]:tile_off[t] + 8 * SD])

                Gt = gp.tile([P, SDP, 128], f16, tag="G")
                c0 = 0
                ic = 0
                GCH = 8           # 1024 descriptors per call (ucode ring limit)
                for g in range(G):
                    D = Ds[g]
                    for d0 in range(0, D, GCH):
                        dn = min(GCH, D - d0)
                        nc.gpsimd.dma_gather(
                            out_ap=Gt[:, c0:c0 + dn, :],
                            in_ap=table[g * GSZ:, :],
                            idxs_ap=it[:, ic:ic + 8 * dn],
                            num_idxs=P * dn,
                            num_idxs_reg=P * dn,
                            elem_size=128,
                            queue_num=nextq(),
                        )
                        c0 += dn
                        ic += 8 * dn
                nc.sync.dma_start(out=Gt[:, SD, :],
                                  in_=ownr[t * P:(t + 1) * P, :])

                # ad_own [p, H] straight from the own-row slot
                adt = sp.tile([P, HH], f16, tag="ad")
                nc.vector.tensor_copy(out=adt[:],
                                      in_=Gt[:, SD, AD_OFF:AD_OFF + HH])

                # nm = -leaky(C + ad_own)   [p, H] f16
                nm = sp.tile([P, HH], f16, tag="nm")
                nc.vector.tensor_add(out=nm[:], in0=adt[:], in1=cb16[:])
                nc.vector.scalar_tensor_tensor(
                    out=nm[:], in0=nm[:], scalar=0.2, in1=nm[:],
                    op0=mybir.AluOpType.mult, op1=mybir.AluOpType.max)
                nc.vector.tensor_scalar_mul(out=nm[:], in0=nm[:], scalar1=-1.0)

                # z = as + ad_own ; L = leaky(z) ; y = L - m' ; ex = exp(y)
                zt = ep.tile([P, SDP, HH], f16, tag="z")
                nc.vector.tensor_tensor(
                    out=zt[:], in0=Gt[:, :, AS_OFF:AS_OFF + HH],
                    in1=adt[:].unsqueeze(1).broadcast_to([P, SDP, HH]),
                    op=mybir.AluOpType.add)
                zf = zt[:].rearrange("p d h -> p (d h)")
                nc.vector.scalar_tensor_tensor(
                    out=zf, in0=zf, scalar=0.2, in1=zf,
                    op0=mybir.AluOpType.mult, op1=mybir.AluOpType.max)
                nc.vector.tensor_tensor(
                    out=zt[:], in0=zt[:],
                    in1=nm[:].unsqueeze(1).broadcast_to([P, SDP, HH]),
                    op=mybir.AluOpType.add)
                ext = ep.tile([P, SDP, HH], f16, tag="ex")
                nc.scalar.activation(
                    out=ext[:].rearrange("p d h -> p (d h)"), in_=zf,
                    func=mybir.ActivationFunctionType.Exp)

                # expand ex -> [p, d, MB] (Act engine)
                exm = xp.tile([P, SDP, MB], f16, tag="exm")
                nc.scalar.activation(
                    out=exm[:, :, 0:CT].rearrange("p d (h c) -> p d h c", h=HH),
                    in_=ext[:].unsqueeze(3).broadcast_to([P, SDP, HH, CC]),
                    func=mybir.ActivationFunctionType.Copy)
                nc.scalar.activation(
                    out=exm[:, :, CT:MB], in_=ext[:],
                    func=mybir.ActivationFunctionType.Copy)

                # msg = feat * ex ; tree-reduce over slots
                mg = mp.tile([P, SDP, MB], f16, tag="mg")
                nc.vector.tensor_tensor(out=mg[:], in0=Gt[:, :, 0:MB],
                                        in1=exm[:], op=mybir.AluOpType.mult)
                cur = SDP
                while cur > 2:
                    half = cur // 2
                    nc.vector.tensor_add(out=mg[:, 0:half],
                                         in0=mg[:, 0:half],
                                         in1=mg[:, cur - half:cur])
                    cur = cur - half
                redf = sp.tile([P, MB], f32, tag="red")
                if cur == 2:
                    nc.vector.tensor_add(out=redf[:], in0=mg[:, 0], in1=mg[:, 1])
                else:
                    nc.vector.tensor_copy(out=redf[:], in_=mg[:, 0])

                # normalize + bias
                rd = sp.tile([P, HH], f32, tag="rd")
                nc.vector.tensor_scalar_add(out=rd[:], in0=redf[:, CT:MB],
                                            scalar1=1e-16)
                nc.vector.reciprocal(out=rd[:], in_=rd[:])
                o1 = sp.tile([P, CT], f32, tag="o1")
                nc.vector.tensor_tensor(
                    out=o1[:].rearrange("p (h c) -> p h c", h=HH),
                    in0=redf[:, 0:CT].rearrange("p (h c) -> p h c", h=HH),
                    in1=rd[:].unsqueeze(2).broadcast_to([P, HH, CC]),
                    op=mybir.AluOpType.mult)
                nc.vector.tensor_add(out=o1[:], in0=o1[:], in1=bt[:])

                if fused:
                    # elu -> fp16
                    t1 = sp.tile([P, CT], f32, tag="t1")
                    nc.vector.tensor_scalar_min(out=t1[:], in0=o1[:], scalar1=0.0)
                    nc.scalar.activation(out=t1[:], in_=t1[:],
                                         func=mybir.ActivationFunctionType.Exp)
                    t2e = sp.tile([P, CT], f32, tag="t2e")
                    nc.vector.tensor_scalar_max(out=t2e[:], in0=o1[:], scalar1=0.0)
                    nc.vector.tensor_add(out=t1[:], in0=t1[:], in1=t2e[:])
                    h16 = sp.tile([P, CT], f16, tag="h16")
                    nc.vector.tensor_scalar_add(out=h16[:], in0=t1[:], scalar1=-1.0)
                    # transpose h16 -> [CT, 128] then @ W2p -> table2 rows
                    psT = pstp.tile([CT, P], f16)
                    nc.tensor.transpose(out=psT[:], in_=h16[:], identity=idt[:])
                    hT = otp.tile([CT, P], f16, tag="hT")
                    nc.vector.tensor_copy(out=hT[:], in_=psT[:])
                    ps2 = ps2p.tile([P, W2RW], f32)
                    nc.tensor.matmul(out=ps2[:], lhsT=hT[:], rhs=w2t[:],
                                     start=True, stop=True)
                    t2t = otp.tile([P, W2RW], f16, tag="t2t")
                    nc.vector.tensor_add(out=t2t[:], in0=ps2[:], in1=ones2[:])
                    nc.sync.dma_start(out=t2o[t * P:(t + 1) * P, :], in_=t2t[:])
                else:
                    nc.sync.dma_start(out=outo[t * P:(t + 1) * P, :], in_=o1[:])

    nc.compile()
    return nc


# --------------------------------------------------------------------------
# host orchestration
# --------------------------------------------------------------------------

def _fold_w1(W1, a_src, a_dst, hh, cc):
    W1r = W1.reshape(W1.shape[0], hh, cc)
    ws = np.einsum("khc,hc->kh", W1r, a_src)
    wd = np.einsum("khc,hc->kh", W1r, a_dst)
    z8 = np.zeros((W1.shape[0], 8), np.float32)
    return np.concatenate([W1, z8, ws, wd, z8], axis=1).astype(np.float16)


def _fold_w2(W2, a_src2, a_dst2):
    z1 = np.zeros((W2.shape[0], 1), np.float32)
    z5 = np.zeros((W2.shape[0], 5), np.float32)
    return np.concatenate([W2, z1, (W2 @ a_src2[0])[:, None],
                           (W2 @ a_dst2[0])[:, None], z5], axis=1).astype(np.float16)


def _get_programs(cfg, D_tbl, tile_off, CB):
    key = (CB, D_tbl.tobytes())
    if key not in _NC_CACHE:
        _NC_CACHE[key] = dict(
            T1=build_transform(cfg),
            E1=build_edge(cfg, 1, D_tbl, tile_off, CB),
            E2=build_edge(cfg, 2, D_tbl, tile_off, CB),
        )
    return _NC_CACHE[key]


def _prep_all(x, edge_index, W1, a_src1, a_dst1, W2, a_src2, a_dst2):
    cfg = make_cfg()
    G, GSZ, NV, SHPAD = cfg["G"], cfg["GSZ"], cfg["NV"], cfg["SHPAD"]
    N = cfg["N_RAW"]
    src, dst = edge_index[0], edge_index[1]
    D_tbl, blobs, orows, tile_off, CB = prep_graph(cfg, src, dst)

    orig = np.arange(N, dtype=np.int64)
    new_id = (orig % G) * GSZ + orig // G

    fT_all = np.zeros((cfg["F_IN"], NV), dtype=np.float16)
    fT_all[:, new_id] = np.asarray(x).T.astype(np.float16)

    W1p = _fold_w1(np.asarray(W1), np.asarray(a_src1), np.asarray(a_dst1),
                   cfg["HH"], cfg["CC"])
    W2p = _fold_w2(np.asarray(W2), np.asarray(a_src2), np.asarray(a_dst2))
    ident = np.eye(P, dtype=np.float16)
    return dict(cfg=cfg, D_tbl=D_tbl, blobs=blobs, orows=orows,
                tile_off=tile_off, CB=CB, new_id=new_id, fT_all=fT_all,
                W1p=W1p, W2p=W2p, ident=ident)


def _run_spmd(nc, in_maps, ncores):
    from concourse.bass_utils import run_bass_kernel_spmd
    res = run_bass_kernel_spmd(nc, in_maps, list(range(ncores)))
    return res.results


def _t1_inmaps(pp):
    cfg = pp["cfg"]
    SHPAD = cfg["SHPAD"]
    return [{"fT": np.ascontiguousarray(pp["fT_all"][:, r * SHPAD:(r + 1) * SHPAD]),
             "Wp": pp["W1p"]} for r in range(cfg["NCORES"])]


def _assemble_table1(pp, outs):
    cfg = pp["cfg"]
    NV, G, GSZ = cfg["NV"], cfg["G"], cfg["GSZ"]
    table1 = np.zeros((NV, 128), dtype=np.float16)
    table1[:, 0:96] = np.vstack([o["tshard"] for o in outs])
    C1 = table1[:, 72:80].astype(np.float32).max(axis=0) + 0.02
    for g in range(G):
        table1[g * GSZ + cfg["PAD_LOCAL"], 72:80] = NEG
    return table1, np.ascontiguousarray(C1, dtype=np.float32)


def _own_rows(pp, table):
    cfg = pp["cfg"]
    rows = []
    for r in range(cfg["NCORES"]):
        v = pp["orows"][r]
        m = v >= 0
        arr = np.zeros((cfg["SHPAD"], 128), np.float16)
        arr[m] = table[pp["new_id"][v[m]]]
        rows.append(arr)
    return rows


def _e1_inmaps(pp, table1, C1, b1):
    cfg = pp["cfg"]
    own = _own_rows(pp, table1)
    return [{"table": table1, "ownr": own[r], "idxb": pp["blobs"][r],
             "bias": np.ascontiguousarray(b1, dtype=np.float32),
             "cbnd": C1, "w2p": pp["W2p"], "ident": pp["ident"]}
            for r in range(cfg["NCORES"])]


def _assemble_table2(pp, outs):
    cfg = pp["cfg"]
    NV, G, GSZ = cfg["NV"], cfg["G"], cfg["GSZ"]
    table2 = np.zeros((NV, 128), dtype=np.float16)
    for r in range(cfg["NCORES"]):
        v = pp["orows"][r]
        m = v >= 0
        table2[pp["new_id"][v[m]], 0:48] = outs[r]["t2o"][m]
    C2 = np.array([table2[:, 41].astype(np.float32).max() + 0.02],
                  dtype=np.float32)
    for g in range(G):
        table2[g * GSZ + cfg["PAD_LOCAL"], 41] = NEG
    return table2, C2


def _e2_inmaps(pp, table2, C2, b2):
    cfg = pp["cfg"]
    own = _own_rows(pp, table2)
    return [{"table": table2, "ownr": own[r], "idxb": pp["blobs"][r],
             "bias": np.ascontiguousarray(b2, dtype=np.float32),
             "cbnd": C2} for r in range(cfg["NCORES"])]


def _final_out(pp, outs):
    cfg = pp["cfg"]
    out = np.empty((cfg["N_RAW"], cfg["OUT_W"]), dtype=np.float32)
    for r in range(cfg["NCORES"]):
        v = pp["orows"][r]
        m = v >= 0
        out[v[m]] = outs[r]["out"][m]
    return out


def kernel(x, edge_index, W1, a_src1, a_dst1, b1, W2, a_src2, a_dst2, b2):
    x = np.asarray(x)
    edge_index = np.asarray(edge_index)
    pp = _prep_all(x, edge_index, W1, a_src1, a_dst1, W2, a_src2, a_dst2)
    cfg = pp["cfg"]
    progs = _get_programs(cfg, pp["D_tbl"], pp["tile_off"], pp["CB"])

    outs = _run_spmd(progs["T1"], _t1_inmaps(pp), cfg["NCORES"])
    table1, C1 = _assemble_table1(pp, outs)

    outs = _run_spmd(progs["E1"], _e1_inmaps(pp, table1, C1, np.asarray(b1)),
                     cfg["NCORES"])
    table2, C2 = _assemble_table2(pp, outs)

    outs = _run_spmd(progs["E2"], _e2_inmaps(pp, table2, C2, np.asarray(b2)),
                     cfg["NCORES"])
    return _final_out(pp, outs)


# --------------------------------------------------------------------------
# timing / profiling helpers (not used by the grader)
# --------------------------------------------------------------------------

def time_launches(inputs, repeats=2, hw=True):
    """Per-launch timing: TimelineSim (cost model) ns + optional HW walls.

    Returns {"T1": [sec...], "E1": [...], "E2": [...]} where values are
    sim exec times in seconds (so test.py's sum(min)*1e9 prints total ns).
    """
    import time as _time
    from concourse.timeline_sim import TimelineSim

    x = np.asarray(inputs["x"])
    ei = np.asarray(inputs["edge_index"])
    pp = _prep_all(x, ei, inputs["W1"], inputs["a_src1"], inputs["a_dst1"],
                   inputs["W2"], inputs["a_src2"], inputs["a_dst2"])
    cfg = pp["cfg"]
    progs = _get_programs(cfg, pp["D_tbl"], pp["tile_off"], pp["CB"])

    times = {}
    for name in ("T1", "E1", "E2"):
        tl = TimelineSim(progs[name], trace=False)
        ns = tl.simulate()
        times[name] = [ns / 1e9]
        print(f"  {name}: sim {ns:.0f} ns", flush=True)

    if hw:
        outs = _run_spmd(progs["T1"], _t1_inmaps(pp), cfg["NCORES"])
        table1, C1 = _assemble_table1(pp, outs)
        e1maps = _e1_inmaps(pp, table1, C1, np.asarray(inputs["b1"]))
        outs = _run_spmd(progs["E1"], e1maps, cfg["NCORES"])
        table2, C2 = _assemble_table2(pp, outs)
        e2maps = _e2_inmaps(pp, table2, C2, np.asarray(inputs["b2"]))
        for name, maps in (("T1", _t1_inmaps(pp)), ("E1", e1maps),
                           ("E2", e2maps)):
            walls = []
            for _ in range(repeats):
                t0 = _time.time()
                _run_spmd(progs[name], maps, cfg["NCORES"])
                walls.append(_time.time() - t0)
            print(f"  {name}: hw walls {[f'{w:.3f}' for w in walls]}", flush=True)
    return times


# revision 46
# speedup vs baseline: 1.0506x; 1.0130x over previous
"""Two-layer GAT on 8 Trainium2 NeuronCores (Bass/Tile, no collectives).

v2 design
---------
* 3 device launches, host does only data marshaling between them:
  - T1: data-parallel transform x@W1p -> table1 shard (each core 1/8 of
    nodes) + per-core max of the a_src scores (softmax bound).
  - E1: edge pass layer 1 (gather + segment softmax + weighted sum) with
    the FULL table1 as a pre-staged input; fuses the layer-2 transform
    (elu(h)@W2p) so its output is directly the table2 shard.
  - E2: edge pass layer 2 -> final [N, 40] rows.
* Node ids relabeled into G=4 groups of GSZ rows (int16 gather indices,
  idx local to group). One pad row per group (a_src = -60000 => exp -> 0).
* Destination nodes are assigned to (core, tile) slots via a Morton-order
  sort of their per-group in-degree profiles: all 8 cores share the same
  per-tile slot counts D_tbl[t,g] (one program for all cores), and Morton
  clustering minimizes sum(max) padding (~1.45x vs 2.13x for degree sort).
* Edge phase per 128-dst tile: dma_gather all incoming rows (one call per
  group, 256B rows = [h | ones | a_s | a_d]), unnormalized-softmax with a
  per-(dst,head) upper bound m' = leaky(C + a_d) so exp stays in fp16,
  denominator via "ones" columns of the same multiply, message sum via
  in-place halving tree adds (packed fp16 -> 2x/4x DVE modes).
"""
import sys
sys.path.insert(0, "/opt/trn_rl_repo")

import numpy as np

P = 128
NEG = -60000.0

_NC_CACHE = {}


def _mybir():
    from concourse import mybir
    return mybir


def make_cfg(n_raw=100000, f_in=512, hh=8, cc=8, out_w=40, ncores=8, gsz=25088):
    g = 4
    sh_pad = 12544                       # 98 tiles * 128 rows per core
    nt = sh_pad // P
    assert ncores * sh_pad == g * gsz    # shard rows == table rows
    return dict(
        N_RAW=n_raw, F_IN=f_in, HH=hh, CC=cc, F_HID=hh * cc, OUT_W=out_w,
        NCORES=ncores, NTILES=nt, SHPAD=sh_pad,
        G=g, GSZ=gsz, PAD_LOCAL=gsz - 1, NV=g * gsz,
    )


# --------------------------------------------------------------------------
# host-side graph prep
# --------------------------------------------------------------------------

def _morton_key(c):
    m = np.zeros(len(c), dtype=np.int64)
    for b in range(6):
        for g in range(c.shape[1]):
            m |= ((c[:, g].astype(np.int64) >> b) & 1) << (b * c.shape[1] + g)
    return m


def prep_graph(cfg, src_orig, dst_orig):
    """Morton-window node placement + per-core gather-index blobs.

    Returns (D_tbl [NT,G], blobs [ncores x [128, CB] int16],
             orows [ncores x [SHPAD] original ids, -1 = empty], tile_off, CB)
    """
    G, GSZ = cfg["G"], cfg["GSZ"]
    NT, SHPAD, NC = cfg["NTILES"], cfg["SHPAD"], cfg["NCORES"]
    N = cfg["N_RAW"]
    PAD_LOCAL = cfg["PAD_LOCAL"]

    src = np.asarray(src_orig, dtype=np.int64)
    dst = np.asarray(dst_orig, dtype=np.int64)

    # per-node per-group edge counts (self-loop handled via direct DMA slot)
    c = np.zeros((N, G), dtype=np.int32)
    np.add.at(c, (dst, src % G), 1)
    own = (np.arange(N) % G).astype(np.int64)

    order = np.argsort(_morton_key(c), kind="stable")
    W = NC * P
    ordp = np.full(NT * W, -1, dtype=np.int64)
    ordp[:N] = order
    owin = ordp.reshape(NT, NC, P)
    orows = [owin[:, r, :].ravel().copy() for r in range(NC)]

    core_of = np.full(N, -1, dtype=np.int64)
    pos_of = np.full(N, -1, dtype=np.int64)
    for r in range(NC):
        v = orows[r]
        m = v >= 0
        core_of[v[m]] = r
        pos_of[v[m]] = np.nonzero(m)[0]

    cnts = []
    per_core_fill = []
    for r in range(NC):
        em = core_of[dst] == r
        es = src[em]
        ep = pos_of[dst[em]]
        sg = es % G
        sl = (es // G).astype(np.int16)
        key = ep * G + sg
        o = np.argsort(key, kind="stable")
        ks, vs = key[o], sl[o]
        cnt = np.bincount(ks, minlength=SHPAD * G).reshape(SHPAD, G)
        starts = np.zeros(SHPAD * G, dtype=np.int64)
        np.cumsum(cnt.ravel()[:-1], out=starts[1:])
        col = np.arange(len(ks)) - starts[ks]
        cnts.append(cnt)
        per_core_fill.append((ks, vs, col))

    cnt_all = np.stack(cnts)                          # [NC, SHPAD, G]
    D_tbl = cnt_all.reshape(NC, NT, P, G).max(axis=(0, 2))      # [NT, G]
    D_tbl = np.maximum(D_tbl, 1)
    dmax = int(D_tbl.max())

    tile_off = []
    off = 0
    for t in range(NT):
        tile_off.append(off)
        off += 8 * int(D_tbl[t].sum())
    CB = off

    blobs = []
    for r in range(NC):
        ks, vs, col = per_core_fill[r]
        big = np.full((SHPAD, G, dmax), PAD_LOCAL, dtype=np.int16)
        big[ks // G, ks % G, col] = vs

        blob = np.empty((P, CB), dtype=np.int16)
        for t in range(NT):
            cpos = tile_off[t]
            for g in range(G):
                D = int(D_tbl[t, g])
                mat = big[t * P:(t + 1) * P, g, :D]      # [128, D]
                L = mat.T.ravel()                         # i = j*128 + p
                W16 = L.reshape(-1, 16).T                 # [16, 8*D]
                blob[:, cpos:cpos + 8 * D] = np.tile(W16, (8, 1))
                cpos += 8 * D
        blobs.append(blob)

    return D_tbl, blobs, orows, tile_off, CB


# --------------------------------------------------------------------------
# device programs
# --------------------------------------------------------------------------

def build_transform(cfg):
    """T1: fT shard [512, SHPAD] @ Wp [512, 96] -> tshard [SHPAD, 96] fp16
    (row = [h64 | ones8 | as8 | ad8 | 0*8]), plus column-max of as -> cmo."""
    import concourse.bass as bass
    import concourse.bacc as bacc
    import concourse.tile as tile
    mybir = _mybir()
    f16, f32 = mybir.dt.float16, mybir.dt.float32

    F_IN, SHPAD, NT = cfg["F_IN"], cfg["SHPAD"], cfg["NTILES"]
    RW = 96

    nc = bacc.Bacc("TRN2", target_bir_lowering=False, debug=False)
    fT = nc.dram_tensor("fT", [F_IN, SHPAD], f16, kind="ExternalInput")
    Wp = nc.dram_tensor("Wp", [F_IN, RW], f16, kind="ExternalInput")
    tshard = nc.dram_tensor("tshard", [SHPAD, RW], f16, kind="ExternalOutput")

    KCH = [(k, min(P, F_IN - k)) for k in range(0, F_IN, P)]

    with tile.TileContext(nc) as tc:
        import contextlib
        with contextlib.ExitStack() as ctx:
            singles = ctx.enter_context(tc.tile_pool(name="singles", bufs=1))
            xtp = ctx.enter_context(tc.tile_pool(name="xt", bufs=3))
            psp = ctx.enter_context(tc.tile_pool(name="ps", bufs=4, space="PSUM"))
            otp = ctx.enter_context(tc.tile_pool(name="ot", bufs=3))
            cp = ctx.enter_context(tc.tile_pool(name="cp", bufs=2))

            wts = []
            for kc, (k0, kn) in enumerate(KCH):
                wt = singles.tile([P, RW], f16, tag=f"w{kc}")
                nc.sync.dma_start(out=wt[:kn, :], in_=Wp[k0:k0 + kn, :])
                wts.append(wt)
            onesrow = singles.tile([P, RW], f32)
            nc.vector.memset(onesrow[:], 0.0)
            nc.vector.memset(onesrow[:, 64:72], 1.0)

            NKC = len(KCH)
            for t0 in range(0, NT, 4):
                nb = min(4, NT - t0)
                xt8 = xtp.tile([P, NKC, nb * P], f16, tag="x8")
                nc.sync.dma_start(
                    out=xt8[:],
                    in_=fT[:, t0 * P:(t0 + nb) * P]
                        .rearrange("(c k) n -> k c n", k=P))
                ot2 = otp.tile([P, nb, RW], f16, tag="ot2")
                for b in range(nb):
                    pt = psp.tile([P, RW], f32)
                    for kc in range(NKC):
                        nc.tensor.matmul(out=pt[:],
                                         lhsT=xt8[:, kc, b * P:(b + 1) * P],
                                         rhs=wts[kc][:],
                                         start=(kc == 0), stop=(kc == NKC - 1))
                    nc.vector.tensor_add(out=ot2[:, b], in0=pt[:], in1=onesrow[:])
                nc.sync.dma_start(
                    out=tshard[t0 * P:(t0 + nb) * P, :]
                        .rearrange("(b r) w -> r b w", r=P),
                    in_=ot2[:])

    nc.compile()
    return nc


def build_edge(cfg, layer, D_tbl, tile_off, CB):
    """Edge pass. layer 1: row [h64|ones8|as8|ad8], fused table2 emit.
    layer 2: row [g40|ones1|as1|ad1], final output rows."""
    import concourse.bass as bass
    import concourse.bacc as bacc
    import concourse.tile as tile
    mybir = _mybir()
    f16, f32, i16 = mybir.dt.float16, mybir.dt.float32, mybir.dt.int16

    G, GSZ, NV, NT = cfg["G"], cfg["GSZ"], cfg["NV"], cfg["NTILES"]
    SHPAD = cfg["SHPAD"]
    HH = cfg["HH"] if layer == 1 else 1
    CC = cfg["CC"] if layer == 1 else cfg["OUT_W"]
    CT = HH * CC
    MB = CT + HH                        # msg block incl ones cols
    AS_OFF, AD_OFF = MB, MB + HH
    OUT_W = CT
    fused = (layer == 1)
    W2RW = 48

    nc = bacc.Bacc("TRN2", target_bir_lowering=False, debug=False,
                   num_swdge_queues=4)
    table = nc.dram_tensor("table", [NV, 128], f16, kind="ExternalInput")
    ownr = nc.dram_tensor("ownr", [SHPAD, 128], f16, kind="ExternalInput")
    idxb = nc.dram_tensor("idxb", [P, CB], i16, kind="ExternalInput")
    bias = nc.dram_tensor("bias", [OUT_W], f32, kind="ExternalInput")
    cbnd = nc.dram_tensor("cbnd", [HH], f32, kind="ExternalInput")
    if fused:
        w2p = nc.dram_tensor("w2p", [CT, W2RW], f16, kind="ExternalInput")
        ident = nc.dram_tensor("ident", [P, P], f16, kind="ExternalInput")
        t2o = nc.dram_tensor("t2o", [SHPAD, W2RW], f16, kind="ExternalOutput")
    else:
        outo = nc.dram_tensor("out", [SHPAD, OUT_W], f32, kind="ExternalOutput")

    qn = [0]

    def nextq():
        q = qn[0]
        qn[0] = (qn[0] + 1) % 4
        return q

    with tile.TileContext(nc) as tc:
        import contextlib
        with contextlib.ExitStack() as ctx:
            singles = ctx.enter_context(tc.tile_pool(name="singles", bufs=1))
            ip = ctx.enter_context(tc.tile_pool(name="ip", bufs=4))
            gp = ctx.enter_context(tc.tile_pool(name="gp", bufs=5))
            ep = ctx.enter_context(tc.tile_pool(name="ep", bufs=3))
            xp = ctx.enter_context(tc.tile_pool(name="xp", bufs=2))
            mp = ctx.enter_context(tc.tile_pool(name="mp", bufs=2))
            sp = ctx.enter_context(tc.tile_pool(name="sp", bufs=4))
            otp = ctx.enter_context(tc.tile_pool(name="otp", bufs=3))
            cp = ctx.enter_context(tc.tile_pool(name="cp", bufs=2))
            if fused:
                pstp = ctx.enter_context(
                    tc.tile_pool(name="pst", bufs=2, space="PSUM"))
                ps2p = ctx.enter_context(
                    tc.tile_pool(name="ps2", bufs=2, space="PSUM"))

            # ---- singles ----
            bias_ap = bias[:]
            bias_b = bass.AP(tensor=bias_ap.tensor, offset=bias_ap.offset,
                             ap=[[0, P]] + list(bias_ap.ap))
            bt = singles.tile([P, OUT_W], f32)
            nc.sync.dma_start(out=bt[:], in_=bias_b)
            cb_ap = cbnd[:]
            cb_b = bass.AP(tensor=cb_ap.tensor, offset=cb_ap.offset,
                           ap=[[0, P]] + list(cb_ap.ap))
            cbf = singles.tile([P, HH], f32)
            nc.sync.dma_start(out=cbf[:], in_=cb_b)
            cb16 = singles.tile([P, HH], f16)
            nc.vector.tensor_copy(out=cb16[:], in_=cbf[:])
            if fused:
                w2t = singles.tile([CT, W2RW], f16)
                nc.sync.dma_start(out=w2t[:], in_=w2p[:, :])
                idt = singles.tile([P, P], f16)
                nc.sync.dma_start(out=idt[:], in_=ident[:, :])
                ones2 = singles.tile([P, W2RW], f32)
                nc.vector.memset(ones2[:], 0.0)
                nc.vector.memset(ones2[:, 40:41], 1.0)

            for t in range(NT):
                Ds = [int(D_tbl[t, g]) for g in range(G)]
                SD = sum(Ds)
                SDP = SD + 1      # + own-row slot (direct DMA, no gather)
                it = ip.tile([P, 8 * SD], i16, tag="idx")
                nc.sync.dma_start(
                    out=it[:], in_=idxb[:, tile_off[t]:tile_off[t] + 8 * SD])

                Gt = gp.tile([P, SDP, 128], f16, tag="G")
                c0 = 0
                ic = 0
                GCH = 8           # 1024 descriptors per call (ucode ring limit)
                for g in range(G):
                    D = Ds[g]
                    for d0 in range(0, D, GCH):
                        dn = min(GCH, D - d0)
                        nc.gpsimd.dma_gather(
                            out_ap=Gt[:, c0:c0 + dn, :],
                            in_ap=table[g * GSZ:, :],
                            idxs_ap=it[:, ic:ic + 8 * dn],
                            num_idxs=P * dn,
                            num_idxs_reg=P * dn,
                            elem_size=128,
                            queue_num=nextq(),
                        )
                        c0 += dn
                        ic += 8 * dn
                nc.sync.dma_start(out=Gt[:, SD, :],
                                  in_=ownr[t * P:(t + 1) * P, :])

                # ad_own [p, H] straight from the own-row slot
                adt = sp.tile([P, HH], f16, tag="ad")
                nc.vector.tensor_copy(out=adt[:],
                                      in_=Gt[:, SD, AD_OFF:AD_OFF + HH])

                # nm = -leaky(C + ad_own)   [p, H] f16
                nm = sp.tile([P, HH], f16, tag="nm")
                nc.vector.tensor_add(out=nm[:], in0=adt[:], in1=cb16[:])
                nc.vector.scalar_tensor_tensor(
                    out=nm[:], in0=nm[:], scalar=0.2, in1=nm[:],
                    op0=mybir.AluOpType.mult, op1=mybir.AluOpType.max)
                nc.vector.tensor_scalar_mul(out=nm[:], in0=nm[:], scalar1=-1.0)

                # z = as + ad_own ; L = leaky(z) ; y = L - m' ; ex = exp(y)
                zt = ep.tile([P, SDP, HH], f16, tag="z")
                nc.vector.tensor_tensor(
                    out=zt[:], in0=Gt[:, :, AS_OFF:AS_OFF + HH],
                    in1=adt[:].unsqueeze(1).broadcast_to([P, SDP, HH]),
                    op=mybir.AluOpType.add)
                zf = zt[:].rearrange("p d h -> p (d h)")
                nc.vector.scalar_tensor_tensor(
                    out=zf, in0=zf, scalar=0.2, in1=zf,
                    op0=mybir.AluOpType.mult, op1=mybir.AluOpType.max)
                nc.vector.tensor_tensor(
                    out=zt[:], in0=zt[:],
                    in1=nm[:].unsqueeze(1).broadcast_to([P, SDP, HH]),
                    op=mybir.AluOpType.add)
                ext = ep.tile([P, SDP, HH], f16, tag="ex")
                nc.scalar.activation(
                    out=ext[:].rearrange("p d h -> p (d h)"), in_=zf,
                    func=mybir.ActivationFunctionType.Exp)

                # expand ex -> [p, d, MB] (Act engine)
                exm = xp.tile([P, SDP, MB], f16, tag="exm")
                nc.scalar.activation(
                    out=exm[:, :, 0:CT].rearrange("p d (h c) -> p d h c", h=HH),
                    in_=ext[:].unsqueeze(3).broadcast_to([P, SDP, HH, CC]),
                    func=mybir.ActivationFunctionType.Copy)
                nc.scalar.activation(
                    out=exm[:, :, CT:MB], in_=ext[:],
                    func=mybir.ActivationFunctionType.Copy)

                # msg = feat * ex ; tree-reduce over slots
                mg = mp.tile([P, SDP, MB], f16, tag="mg")
                nc.vector.tensor_tensor(out=mg[:], in0=Gt[:, :, 0:MB],
                                        in1=exm[:], op=mybir.AluOpType.mult)
                cur = SDP
                while cur > 2:
                    half = cur // 2
                    nc.vector.tensor_add(out=mg[:, 0:half],
                                         in0=mg[:, 0:half],
                                         in1=mg[:, cur - half:cur])
                    cur = cur - half
                redf = sp.tile([P, MB], f32, tag="red")
                if cur == 2:
                    nc.vector.tensor_add(out=redf[:], in0=mg[:, 0], in1=mg[:, 1])
                else:
                    nc.vector.tensor_copy(out=redf[:], in_=mg[:, 0])

                # normalize + bias
                rd = sp.tile([P, HH], f32, tag="rd")
                nc.vector.tensor_scalar_add(out=rd[:], in0=redf[:, CT:MB],
                                            scalar1=1e-16)
                nc.vector.reciprocal(out=rd[:], in_=rd[:])
                o1 = sp.tile([P, CT], f32, tag="o1")
                nc.vector.tensor_tensor(
                    out=o1[:].rearrange("p (h c) -> p h c", h=HH),
                    in0=redf[:, 0:CT].rearrange("p (h c) -> p h c", h=HH),
                    in1=rd[:].unsqueeze(2).broadcast_to([P, HH, CC]),
                    op=mybir.AluOpType.mult)
                nc.vector.tensor_add(out=o1[:], in0=o1[:], in1=bt[:])

                if fused:
                    # elu -> fp16
                    t1 = sp.tile([P, CT], f32, tag="t1")
                    nc.vector.tensor_scalar_min(out=t1[:], in0=o1[:], scalar1=0.0)
                    nc.scalar.activation(out=t1[:], in_=t1[:],
                                         func=mybir.ActivationFunctionType.Exp)
                    t2e = sp.tile([P, CT], f32, tag="t2e")
                    nc.vector.tensor_scalar_max(out=t2e[:], in0=o1[:], scalar1=0.0)
                    nc.vector.tensor_add(out=t1[:], in0=t1[:], in1=t2e[:])
                    h16 = sp.tile([P, CT], f16, tag="h16")
                    nc.vector.tensor_scalar_add(out=h16[:], in0=t1[:], scalar1=-1.0)
                    # transpose h16 -> [CT, 128] then @ W2p -> table2 rows
                    psT = pstp.tile([CT, P], f16)
                    nc.tensor.transpose(out=psT[:], in_=h16[:], identity=idt[:])
                    hT = otp.tile([CT, P], f16, tag="hT")
                    nc.vector.tensor_copy(out=hT[:], in_=psT[:])
                    ps2 = ps2p.tile([P, W2RW], f32)
                    nc.tensor.matmul(out=ps2[:], lhsT=hT[:], rhs=w2t[:],
                                     start=True, stop=True)
                    t2t = otp.tile([P, W2RW], f16, tag="t2t")
                    nc.vector.tensor_add(out=t2t[:], in0=ps2[:], in1=ones2[:])
                    nc.sync.dma_start(out=t2o[t * P:(t + 1) * P, :], in_=t2t[:])
                else:
                    nc.sync.dma_start(out=outo[t * P:(t + 1) * P, :], in_=o1[:])

    nc.compile()
    return nc


# --------------------------------------------------------------------------
# host orchestration
# --------------------------------------------------------------------------

def _fold_w1(W1, a_src, a_dst, hh, cc):
    W1r = W1.reshape(W1.shape[0], hh, cc)
    ws = np.einsum("khc,hc->kh", W1r, a_src)
    wd = np.einsum("khc,hc->kh", W1r, a_dst)
    z8 = np.zeros((W1.shape[0], 8), np.float32)
    return np.concatenate([W1, z8, ws, wd, z8], axis=1).astype(np.float16)


def _fold_w2(W2, a_src2, a_dst2):
    z1 = np.zeros((W2.shape[0], 1), np.float32)
    z5 = np.zeros((W2.shape[0], 5), np.float32)
    return np.concatenate([W2, z1, (W2 @ a_src2[0])[:, None],
                           (W2 @ a_dst2[0])[:, None], z5], axis=1).astype(np.float16)


def _get_programs(cfg, D_tbl, tile_off, CB):
    key = (CB, D_tbl.tobytes())
    if key not in _NC_CACHE:
        _NC_CACHE[key] = dict(
            T1=build_transform(cfg),
            E1=build_edge(cfg, 1, D_tbl, tile_off, CB),
            E2=build_edge(cfg, 2, D_tbl, tile_off, CB),
        )
    return _NC_CACHE[key]


def _prep_all(x, edge_index, W1, a_src1, a_dst1, W2, a_src2, a_dst2):
    cfg = make_cfg()
    G, GSZ, NV, SHPAD = cfg["G"], cfg["GSZ"], cfg["NV"], cfg["SHPAD"]
    N = cfg["N_RAW"]
    src, dst = edge_index[0], edge_index[1]
    D_tbl, blobs, orows, tile_off, CB = prep_graph(cfg, src, dst)

    orig = np.arange(N, dtype=np.int64)
    new_id = (orig % G) * GSZ + orig // G

    fT_all = np.zeros((cfg["F_IN"], NV), dtype=np.float16)
    fT_all[:, new_id] = np.asarray(x).T.astype(np.float16)

    W1p = _fold_w1(np.asarray(W1), np.asarray(a_src1), np.asarray(a_dst1),
                   cfg["HH"], cfg["CC"])
    W2p = _fold_w2(np.asarray(W2), np.asarray(a_src2), np.asarray(a_dst2))
    ident = np.eye(P, dtype=np.float16)
    return dict(cfg=cfg, D_tbl=D_tbl, blobs=blobs, orows=orows,
                tile_off=tile_off, CB=CB, new_id=new_id, fT_all=fT_all,
                W1p=W1p, W2p=W2p, ident=ident)


def _run_spmd(nc, in_maps, ncores):
    from concourse.bass_utils import run_bass_kernel_spmd
    res = run_bass_kernel_spmd(nc, in_maps, list(range(ncores)))
    return res.results


def _t1_inmaps(pp):
    cfg = pp["cfg"]
    SHPAD = cfg["SHPAD"]
    return [{"fT": np.ascontiguousarray(pp["fT_all"][:, r * SHPAD:(r + 1) * SHPAD]),
             "Wp": pp["W1p"]} for r in range(cfg["NCORES"])]


def _assemble_table1(pp, outs):
    cfg = pp["cfg"]
    NV, G, GSZ = cfg["NV"], cfg["G"], cfg["GSZ"]
    table1 = np.zeros((NV, 128), dtype=np.float16)
    table1[:, 0:96] = np.vstack([o["tshard"] for o in outs])
    C1 = table1[:, 72:80].astype(np.float32).max(axis=0) + 0.02
    for g in range(G):
        table1[g * GSZ + cfg["PAD_LOCAL"], 72:80] = NEG
    return table1, np.ascontiguousarray(C1, dtype=np.float32)


def _own_rows(pp, table):
    cfg = pp["cfg"]
    rows = []
    for r in range(cfg["NCORES"]):
        v = pp["orows"][r]
        m = v >= 0
        arr = np.zeros((cfg["SHPAD"], 128), np.float16)
        arr[m] = table[pp["new_id"][v[m]]]
        rows.append(arr)
    return rows


def _e1_inmaps(pp, table1, C1, b1):
    cfg = pp["cfg"]
    own = _own_rows(pp, table1)
    return [{"table": table1, "ownr": own[r], "idxb": pp["blobs"][r],
             "bias": np.ascontiguousarray(b1, dtype=np.float32),
             "cbnd": C1, "w2p": pp["W2p"], "ident": pp["ident"]}
            for r in range(cfg["NCORES"])]


def _assemble_table2(pp, outs):
    cfg = pp["cfg"]
    NV, G, GSZ = cfg["NV"], cfg["G"], cfg["GSZ"]
    table2 = np.zeros((NV, 128), dtype=np.float16)
    for r in range(cfg["NCORES"]):
        v = pp["orows"][r]
        m = v >= 0
        table2[pp["new_id"][v[m]], 0:48] = outs[r]["t2o"][m]
    C2 = np.array([table2[:, 41].astype(np.float32).max() + 0.02],
                  dtype=np.float32)
    for g in range(G):
        table2[g * GSZ + cfg["PAD_LOCAL"], 41] = NEG
    return table2, C2


def _e2_inmaps(pp, table2, C2, b2):
    cfg = pp["cfg"]
    own = _own_rows(pp, table2)
    return [{"table": table2, "ownr": own[r], "idxb": pp["blobs"][r],
             "bias": np.ascontiguousarray(b2, dtype=np.float32),
             "cbnd": C2} for r in range(cfg["NCORES"])]


def _final_out(pp, outs):
    cfg = pp["cfg"]
    out = np.empty((cfg["N_RAW"], cfg["OUT_W"]), dtype=np.float32)
    for r in range(cfg["NCORES"]):
        v = pp["orows"][r]
        m = v >= 0
        out[v[m]] = outs[r]["out"][m]
    return out


def kernel(x, edge_index, W1, a_src1, a_dst1, b1, W2, a_src2, a_dst2, b2):
    x = np.asarray(x)
    edge_index = np.asarray(edge_index)
    pp = _prep_all(x, edge_index, W1, a_src1, a_dst1, W2, a_src2, a_dst2)
    cfg = pp["cfg"]
    progs = _get_programs(cfg, pp["D_tbl"], pp["tile_off"], pp["CB"])

    outs = _run_spmd(progs["T1"], _t1_inmaps(pp), cfg["NCORES"])
    table1, C1 = _assemble_table1(pp, outs)

    outs = _run_spmd(progs["E1"], _e1_inmaps(pp, table1, C1, np.asarray(b1)),
                     cfg["NCORES"])
    table2, C2 = _assemble_table2(pp, outs)

    outs = _run_spmd(progs["E2"], _e2_inmaps(pp, table2, C2, np.asarray(b2)),
                     cfg["NCORES"])
    return _final_out(pp, outs)


# --------------------------------------------------------------------------
# timing / profiling helpers (not used by the grader)
# --------------------------------------------------------------------------

def time_launches(inputs, repeats=2, hw=True):
    """Per-launch timing: TimelineSim (cost model) ns + optional HW walls.

    Returns {"T1": [sec...], "E1": [...], "E2": [...]} where values are
    sim exec times in seconds (so test.py's sum(min)*1e9 prints total ns).
    """
    import time as _time
    from concourse.timeline_sim import TimelineSim

    x = np.asarray(inputs["x"])
    ei = np.asarray(inputs["edge_index"])
    pp = _prep_all(x, ei, inputs["W1"], inputs["a_src1"], inputs["a_dst1"],
                   inputs["W2"], inputs["a_src2"], inputs["a_dst2"])
    cfg = pp["cfg"]
    progs = _get_programs(cfg, pp["D_tbl"], pp["tile_off"], pp["CB"])

    times = {}
    for name in ("T1", "E1", "E2"):
        tl = TimelineSim(progs[name], trace=False)
        ns = tl.simulate()
        times[name] = [ns / 1e9]
        print(f"  {name}: sim {ns:.0f} ns", flush=True)

    if hw:
        outs = _run_spmd(progs["T1"], _t1_inmaps(pp), cfg["NCORES"])
        table1, C1 = _assemble_table1(pp, outs)
        e1maps = _e1_inmaps(pp, table1, C1, np.asarray(inputs["b1"]))
        outs = _run_spmd(progs["E1"], e1maps, cfg["NCORES"])
        table2, C2 = _assemble_table2(pp, outs)
        e2maps = _e2_inmaps(pp, table2, C2, np.asarray(inputs["b2"]))
        for name, maps in (("T1", _t1_inmaps(pp)), ("E1", e1maps),
                           ("E2", e2maps)):
            walls = []
            for _ in range(repeats):
                t0 = _time.time()
                _run_spmd(progs[name], maps, cfg["NCORES"])
                walls.append(_time.time() - t0)
            print(f"  {name}: hw walls {[f'{w:.3f}' for w in walls]}", flush=True)
    return times


# revision 50
# speedup vs baseline: 1.0661x; 1.0148x over previous
"""Two-layer GAT on 8 Trainium2 NeuronCores (Bass/Tile, no collectives).

v2 design
---------
* 3 device launches, host does only data marshaling between them:
  - T1: data-parallel transform x@W1p -> table1 shard (each core 1/8 of
    nodes); 4 tiles batched per DMA (HWDGE-gen bound otherwise).
  - E1: edge pass layer 1 (gather + segment softmax + weighted sum) with
    the FULL table1 as a pre-staged input; fuses the layer-2 transform
    (elu(h)@W2p via PE transpose + matmul) so its output is directly the
    table2 shard.
  - E2: edge pass layer 2 -> final [N, 40] rows.
  Softmax bounds C (max a_src over nodes) come free on the host from the
  previous launch's output.
* Node ids relabeled into G=4 groups of GSZ rows (int16 gather indices,
  idx local to group). One pad row per group (a_src = -60000 => exp -> 0).
* Destination nodes are assigned to (core, tile) slots via a Morton-order
  sort of their per-group in-degree profiles: all 8 cores share the same
  per-tile slot counts D_tbl[t,g] (one program for all cores), and Morton
  clustering minimizes sum(max) padding (~1.38x vs 2.13x for degree sort).
  Each tile's own row arrives by direct DMA (extra slot), not gather, so
  no gather slots are reserved for self-loops / a_d extraction.
* Edge phase per 128-dst tile: dma_gather all incoming rows (<=8 slots =
  1024 descriptors per call -- the SWDGE ucode ring limit; 4 queues
  round-robin; 256B rows = [h | ones | a_s | a_d]), unnormalized-softmax
  with a per-(dst,head) upper bound m' = leaky(C + a_d) so exp stays in
  fp16, denominator via "ones" columns of the same multiply, message sum
  via in-place halving tree adds (packed fp16 -> 2x DVE mode), exp and
  ex-broadcast on the Act engine.
"""
import sys
sys.path.insert(0, "/opt/trn_rl_repo")

import numpy as np

P = 128
NEG = -60000.0

_NC_CACHE = {}


def _mybir():
    from concourse import mybir
    return mybir


def make_cfg(n_raw=100000, f_in=512, hh=8, cc=8, out_w=40, ncores=8, gsz=25088):
    g = 4
    sh_pad = 12544                       # 98 tiles * 128 rows per core
    nt = sh_pad // P
    assert ncores * sh_pad == g * gsz    # shard rows == table rows
    return dict(
        N_RAW=n_raw, F_IN=f_in, HH=hh, CC=cc, F_HID=hh * cc, OUT_W=out_w,
        NCORES=ncores, NTILES=nt, SHPAD=sh_pad,
        G=g, GSZ=gsz, PAD_LOCAL=gsz - 1, NV=g * gsz,
    )


# --------------------------------------------------------------------------
# host-side graph prep
# --------------------------------------------------------------------------

def _morton_key(c):
    m = np.zeros(len(c), dtype=np.int64)
    for b in range(6):
        for g in range(c.shape[1]):
            m |= ((c[:, g].astype(np.int64) >> b) & 1) << (b * c.shape[1] + g)
    return m


def prep_graph(cfg, src_orig, dst_orig):
    """Morton-window node placement + per-core gather-index blobs.

    Returns (D_tbl [NT,G], blobs [ncores x [128, CB] int16],
             orows [ncores x [SHPAD] original ids, -1 = empty], tile_off, CB)
    """
    G, GSZ = cfg["G"], cfg["GSZ"]
    NT, SHPAD, NC = cfg["NTILES"], cfg["SHPAD"], cfg["NCORES"]
    N = cfg["N_RAW"]
    PAD_LOCAL = cfg["PAD_LOCAL"]

    src = np.asarray(src_orig, dtype=np.int64)
    dst = np.asarray(dst_orig, dtype=np.int64)

    # per-node per-group edge counts (self-loop handled via direct DMA slot)
    c = np.zeros((N, G), dtype=np.int32)
    np.add.at(c, (dst, src % G), 1)
    own = (np.arange(N) % G).astype(np.int64)

    order = np.argsort(_morton_key(c), kind="stable")
    W = NC * P
    ordp = np.full(NT * W, -1, dtype=np.int64)
    ordp[:N] = order
    owin = ordp.reshape(NT, NC, P)
    orows = [owin[:, r, :].ravel().copy() for r in range(NC)]

    core_of = np.full(N, -1, dtype=np.int64)
    pos_of = np.full(N, -1, dtype=np.int64)
    for r in range(NC):
        v = orows[r]
        m = v >= 0
        core_of[v[m]] = r
        pos_of[v[m]] = np.nonzero(m)[0]

    cnts = []
    per_core_fill = []
    for r in range(NC):
        em = core_of[dst] == r
        es = src[em]
        ep = pos_of[dst[em]]
        sg = es % G
        sl = (es // G).astype(np.int16)
        key = ep * G + sg
        o = np.argsort(key, kind="stable")
        ks, vs = key[o], sl[o]
        cnt = np.bincount(ks, minlength=SHPAD * G).reshape(SHPAD, G)
        starts = np.zeros(SHPAD * G, dtype=np.int64)
        np.cumsum(cnt.ravel()[:-1], out=starts[1:])
        col = np.arange(len(ks)) - starts[ks]
        cnts.append(cnt)
        per_core_fill.append((ks, vs, col))

    cnt_all = np.stack(cnts)                          # [NC, SHPAD, G]
    D_tbl = cnt_all.reshape(NC, NT, P, G).max(axis=(0, 2))      # [NT, G]
    D_tbl = np.maximum(D_tbl, 1)
    dmax = int(D_tbl.max())

    tile_off = []
    off = 0
    for t in range(NT):
        tile_off.append(off)
        off += 8 * int(D_tbl[t].sum()) + 128   # +128: own-row bytes slot
    CB = off

    blobs = []
    for r in range(NC):
        ks, vs, col = per_core_fill[r]
        big = np.full((SHPAD, G, dmax), PAD_LOCAL, dtype=np.int16)
        big[ks // G, ks % G, col] = vs

        blob = np.empty((P, CB), dtype=np.int16)
        for t in range(NT):
            cpos = tile_off[t]
            for g in range(G):
                D = int(D_tbl[t, g])
                mat = big[t * P:(t + 1) * P, g, :D]      # [128, D]
                L = mat.T.ravel()                         # i = j*128 + p
                W16 = L.reshape(-1, 16).T                 # [16, 8*D]
                blob[:, cpos:cpos + 8 * D] = np.tile(W16, (8, 1))
                cpos += 8 * D
        blobs.append(blob)

    return D_tbl, blobs, orows, tile_off, CB


# --------------------------------------------------------------------------
# device programs
# --------------------------------------------------------------------------

def build_transform(cfg):
    """T1: fT shard [512, SHPAD] @ Wp [512, 96] -> tshard [SHPAD, 96] fp16
    (row = [h64 | ones8 | as8 | ad8 | 0*8]), plus column-max of as -> cmo."""
    import concourse.bass as bass
    import concourse.bacc as bacc
    import concourse.tile as tile
    mybir = _mybir()
    f16, f32 = mybir.dt.float16, mybir.dt.float32

    F_IN, SHPAD, NT = cfg["F_IN"], cfg["SHPAD"], cfg["NTILES"]
    RW = 96

    nc = bacc.Bacc("TRN2", target_bir_lowering=False, debug=False)
    fT = nc.dram_tensor("fT", [F_IN, SHPAD], f16, kind="ExternalInput")
    Wp = nc.dram_tensor("Wp", [F_IN, RW], f16, kind="ExternalInput")
    tshard = nc.dram_tensor("tshard", [SHPAD, RW], f16, kind="ExternalOutput")

    KCH = [(k, min(P, F_IN - k)) for k in range(0, F_IN, P)]

    with tile.TileContext(nc) as tc:
        import contextlib
        with contextlib.ExitStack() as ctx:
            singles = ctx.enter_context(tc.tile_pool(name="singles", bufs=1))
            xtp = ctx.enter_context(tc.tile_pool(name="xt", bufs=3))
            psp = ctx.enter_context(tc.tile_pool(name="ps", bufs=4, space="PSUM"))
            otp = ctx.enter_context(tc.tile_pool(name="ot", bufs=3))
            cp = ctx.enter_context(tc.tile_pool(name="cp", bufs=2))

            wts = []
            for kc, (k0, kn) in enumerate(KCH):
                wt = singles.tile([P, RW], f16, tag=f"w{kc}")
                nc.sync.dma_start(out=wt[:kn, :], in_=Wp[k0:k0 + kn, :])
                wts.append(wt)
            onesrow = singles.tile([P, RW], f32)
            nc.vector.memset(onesrow[:], 0.0)
            nc.vector.memset(onesrow[:, 64:72], 1.0)

            NKC = len(KCH)
            for t0 in range(0, NT, 4):
                nb = min(4, NT - t0)
                xt8 = xtp.tile([P, NKC, nb * P], f16, tag="x8")
                nc.sync.dma_start(
                    out=xt8[:],
                    in_=fT[:, t0 * P:(t0 + nb) * P]
                        .rearrange("(c k) n -> k c n", k=P))
                ot2 = otp.tile([P, nb, RW], f16, tag="ot2")
                for b in range(nb):
                    pt = psp.tile([P, RW], f32)
                    for kc in range(NKC):
                        nc.tensor.matmul(out=pt[:],
                                         lhsT=xt8[:, kc, b * P:(b + 1) * P],
                                         rhs=wts[kc][:],
                                         start=(kc == 0), stop=(kc == NKC - 1))
                    nc.vector.tensor_add(out=ot2[:, b], in0=pt[:], in1=onesrow[:])
                nc.sync.dma_start(
                    out=tshard[t0 * P:(t0 + nb) * P, :]
                        .rearrange("(b r) w -> r b w", r=P),
                    in_=ot2[:])

    nc.compile()
    return nc


def build_edge(cfg, layer, D_tbl, tile_off, CB):
    """Edge pass. layer 1: row [h64|ones8|as8|ad8], fused table2 emit.
    layer 2: row [g40|ones1|as1|ad1], final output rows."""
    import concourse.bass as bass
    import concourse.bacc as bacc
    import concourse.tile as tile
    mybir = _mybir()
    f16, f32, i16 = mybir.dt.float16, mybir.dt.float32, mybir.dt.int16

    G, GSZ, NV, NT = cfg["G"], cfg["GSZ"], cfg["NV"], cfg["NTILES"]
    SHPAD = cfg["SHPAD"]
    HH = cfg["HH"] if layer == 1 else 1
    CC = cfg["CC"] if layer == 1 else cfg["OUT_W"]
    CT = HH * CC
    MB = CT + HH                        # msg block incl ones cols
    AS_OFF, AD_OFF = MB, MB + HH
    OUT_W = CT
    fused = (layer == 1)
    W2RW = 48

    nc = bacc.Bacc("TRN2", target_bir_lowering=False, debug=False,
                   num_swdge_queues=4)
    table = nc.dram_tensor("table", [NV, 128], f16, kind="ExternalInput")
    idxb = nc.dram_tensor("idxb", [P, CB], i16, kind="ExternalInput")
    bias = nc.dram_tensor("bias", [OUT_W], f32, kind="ExternalInput")
    cbnd = nc.dram_tensor("cbnd", [HH], f32, kind="ExternalInput")
    if fused:
        w2p = nc.dram_tensor("w2p", [CT, W2RW], f16, kind="ExternalInput")
        ident = nc.dram_tensor("ident", [P, P], f16, kind="ExternalInput")
        t2o = nc.dram_tensor("t2o", [SHPAD, W2RW], f16, kind="ExternalOutput")
    else:
        outo = nc.dram_tensor("out", [SHPAD, OUT_W], f32, kind="ExternalOutput")

    qn = [0]

    def nextq():
        q = qn[0]
        qn[0] = (qn[0] + 1) % 4
        return q

    with tile.TileContext(nc) as tc:
        import contextlib
        with contextlib.ExitStack() as ctx:
            singles = ctx.enter_context(tc.tile_pool(name="singles", bufs=1))
            ip = ctx.enter_context(tc.tile_pool(name="ip", bufs=4))
            gp = ctx.enter_context(tc.tile_pool(name="gp", bufs=5))
            ep = ctx.enter_context(tc.tile_pool(name="ep", bufs=3))
            xp = ctx.enter_context(tc.tile_pool(name="xp", bufs=2))
            mp = ctx.enter_context(tc.tile_pool(name="mp", bufs=2))
            sp = ctx.enter_context(tc.tile_pool(name="sp", bufs=4))
            otp = ctx.enter_context(tc.tile_pool(name="otp", bufs=3))
            cp = ctx.enter_context(tc.tile_pool(name="cp", bufs=2))
            if fused:
                pstp = ctx.enter_context(
                    tc.tile_pool(name="pst", bufs=2, space="PSUM"))
                ps2p = ctx.enter_context(
                    tc.tile_pool(name="ps2", bufs=2, space="PSUM"))

            # ---- singles ----
            bias_ap = bias[:]
            bias_b = bass.AP(tensor=bias_ap.tensor, offset=bias_ap.offset,
                             ap=[[0, P]] + list(bias_ap.ap))
            bt = singles.tile([P, OUT_W], f32)
            nc.sync.dma_start(out=bt[:], in_=bias_b)
            cb_ap = cbnd[:]
            cb_b = bass.AP(tensor=cb_ap.tensor, offset=cb_ap.offset,
                           ap=[[0, P]] + list(cb_ap.ap))
            cbf = singles.tile([P, HH], f32)
            nc.sync.dma_start(out=cbf[:], in_=cb_b)
            cb16 = singles.tile([P, HH], f16)
            nc.vector.tensor_copy(out=cb16[:], in_=cbf[:])
            if fused:
                w2t = singles.tile([CT, W2RW], f16)
                nc.sync.dma_start(out=w2t[:], in_=w2p[:, :])
                idt = singles.tile([P, P], f16)
                nc.sync.dma_start(out=idt[:], in_=ident[:, :])
                ones2 = singles.tile([P, W2RW], f32)
                nc.vector.memset(ones2[:], 0.0)
                nc.vector.memset(ones2[:, 40:41], 1.0)

            for t in range(NT):
                Ds = [int(D_tbl[t, g]) for g in range(G)]
                SD = sum(Ds)
                SDP = SD + 1      # + own-row slot (direct DMA, no gather)
                it = ip.tile([P, 8 * SD + 128], i16, tag="idx")
                nc.sync.dma_start(
                    out=it[:],
                    in_=idxb[:, tile_off[t]:tile_off[t] + 8 * SD + 128])

                Gt = gp.tile([P, SDP, 128], f16, tag="G")
                c0 = 0
                ic = 0
                GCH = 8           # 1024 descriptors per call (ucode ring limit)
                for g in range(G):
                    D = Ds[g]
                    for d0 in range(0, D, GCH):
                        dn = min(GCH, D - d0)
                        nc.gpsimd.dma_gather(
                            out_ap=Gt[:, c0:c0 + dn, :],
                            in_ap=table[g * GSZ:, :],
                            idxs_ap=it[:, ic:ic + 8 * dn],
                            num_idxs=P * dn,
                            num_idxs_reg=P * dn,
                            elem_size=128,
                            queue_num=nextq(),
                        )
                        c0 += dn
                        ic += 8 * dn
                nc.vector.tensor_copy(
                    out=Gt[:, SD, :],
                    in_=it[:, 8 * SD:8 * SD + 128].bitcast(f16))

                # ad_own [p, H] straight from the own-row slot
                adt = sp.tile([P, HH], f16, tag="ad")
                nc.vector.tensor_copy(out=adt[:],
                                      in_=Gt[:, SD, AD_OFF:AD_OFF + HH])

                # nm = -leaky(C + ad_own)   [p, H] f16
                nm = sp.tile([P, HH], f16, tag="nm")
                nc.vector.tensor_add(out=nm[:], in0=adt[:], in1=cb16[:])
                nc.vector.scalar_tensor_tensor(
                    out=nm[:], in0=nm[:], scalar=0.2, in1=nm[:],
                    op0=mybir.AluOpType.mult, op1=mybir.AluOpType.max)
                nc.vector.tensor_scalar_mul(out=nm[:], in0=nm[:], scalar1=-1.0)

                # z = as + ad_own ; L = leaky(z) ; y = L - m' ; ex = exp(y)
                zt = ep.tile([P, SDP, HH], f16, tag="z")
                nc.vector.tensor_tensor(
                    out=zt[:], in0=Gt[:, :, AS_OFF:AS_OFF + HH],
                    in1=adt[:].unsqueeze(1).broadcast_to([P, SDP, HH]),
                    op=mybir.AluOpType.add)
                zf = zt[:].rearrange("p d h -> p (d h)")
                nc.vector.scalar_tensor_tensor(
                    out=zf, in0=zf, scalar=0.2, in1=zf,
                    op0=mybir.AluOpType.mult, op1=mybir.AluOpType.max)
                nc.vector.tensor_tensor(
                    out=zt[:], in0=zt[:],
                    in1=nm[:].unsqueeze(1).broadcast_to([P, SDP, HH]),
                    op=mybir.AluOpType.add)
                ext = ep.tile([P, SDP, HH], f16, tag="ex")
                nc.scalar.activation(
                    out=ext[:].rearrange("p d h -> p (d h)"), in_=zf,
                    func=mybir.ActivationFunctionType.Exp)

                # expand ex -> [p, d, MB] (Act engine)
                exm = xp.tile([P, SDP, MB], f16, tag="exm")
                nc.scalar.activation(
                    out=exm[:, :, 0:CT].rearrange("p d (h c) -> p d h c", h=HH),
                    in_=ext[:].unsqueeze(3).broadcast_to([P, SDP, HH, CC]),
                    func=mybir.ActivationFunctionType.Copy)
                nc.scalar.activation(
                    out=exm[:, :, CT:MB], in_=ext[:],
                    func=mybir.ActivationFunctionType.Copy)

                # msg = feat * ex ; tree-reduce over slots
                mg = mp.tile([P, SDP, MB], f16, tag="mg")
                nc.vector.tensor_tensor(out=mg[:], in0=Gt[:, :, 0:MB],
                                        in1=exm[:], op=mybir.AluOpType.mult)
                cur = SDP
                while cur > 2:
                    half = cur // 2
                    nc.vector.tensor_add(out=mg[:, 0:half],
                                         in0=mg[:, 0:half],
                                         in1=mg[:, cur - half:cur])
                    cur = cur - half
                redf = sp.tile([P, MB], f32, tag="red")
                if cur == 2:
                    nc.vector.tensor_add(out=redf[:], in0=mg[:, 0], in1=mg[:, 1])
                else:
                    nc.vector.tensor_copy(out=redf[:], in_=mg[:, 0])

                # normalize + bias
                rd = sp.tile([P, HH], f32, tag="rd")
                nc.vector.tensor_scalar_add(out=rd[:], in0=redf[:, CT:MB],
                                            scalar1=1e-16)
                nc.vector.reciprocal(out=rd[:], in_=rd[:])
                o1 = sp.tile([P, CT], f32, tag="o1")
                nc.vector.tensor_tensor(
                    out=o1[:].rearrange("p (h c) -> p h c", h=HH),
                    in0=redf[:, 0:CT].rearrange("p (h c) -> p h c", h=HH),
                    in1=rd[:].unsqueeze(2).broadcast_to([P, HH, CC]),
                    op=mybir.AluOpType.mult)
                nc.vector.tensor_add(out=o1[:], in0=o1[:], in1=bt[:])

                if fused:
                    # elu -> fp16
                    t1 = sp.tile([P, CT], f32, tag="t1")
                    nc.vector.tensor_scalar_min(out=t1[:], in0=o1[:], scalar1=0.0)
                    nc.scalar.activation(out=t1[:], in_=t1[:],
                                         func=mybir.ActivationFunctionType.Exp)
                    t2e = sp.tile([P, CT], f32, tag="t2e")
                    nc.vector.tensor_scalar_max(out=t2e[:], in0=o1[:], scalar1=0.0)
                    nc.vector.tensor_add(out=t1[:], in0=t1[:], in1=t2e[:])
                    h16 = sp.tile([P, CT], f16, tag="h16")
                    nc.vector.tensor_scalar_add(out=h16[:], in0=t1[:], scalar1=-1.0)
                    # transpose h16 -> [CT, 128] then @ W2p -> table2 rows
                    psT = pstp.tile([CT, P], f16)
                    nc.tensor.transpose(out=psT[:], in_=h16[:], identity=idt[:])
                    hT = otp.tile([CT, P], f16, tag="hT")
                    nc.vector.tensor_copy(out=hT[:], in_=psT[:])
                    ps2 = ps2p.tile([P, W2RW], f32)
                    nc.tensor.matmul(out=ps2[:], lhsT=hT[:], rhs=w2t[:],
                                     start=True, stop=True)
                    t2t = otp.tile([P, W2RW], f16, tag="t2t")
                    nc.vector.tensor_add(out=t2t[:], in0=ps2[:], in1=ones2[:])
                    nc.sync.dma_start(out=t2o[t * P:(t + 1) * P, :], in_=t2t[:])
                else:
                    nc.sync.dma_start(out=outo[t * P:(t + 1) * P, :], in_=o1[:])

    nc.compile()
    return nc


# --------------------------------------------------------------------------
# host orchestration
# --------------------------------------------------------------------------

def _fold_w1(W1, a_src, a_dst, hh, cc):
    W1r = W1.reshape(W1.shape[0], hh, cc)
    ws = np.einsum("khc,hc->kh", W1r, a_src)
    wd = np.einsum("khc,hc->kh", W1r, a_dst)
    z8 = np.zeros((W1.shape[0], 8), np.float32)
    return np.concatenate([W1, z8, ws, wd, z8], axis=1).astype(np.float16)


def _fold_w2(W2, a_src2, a_dst2):
    z1 = np.zeros((W2.shape[0], 1), np.float32)
    z5 = np.zeros((W2.shape[0], 5), np.float32)
    return np.concatenate([W2, z1, (W2 @ a_src2[0])[:, None],
                           (W2 @ a_dst2[0])[:, None], z5], axis=1).astype(np.float16)


def _get_programs(cfg, D_tbl, tile_off, CB):
    key = (CB, D_tbl.tobytes())
    if key not in _NC_CACHE:
        _NC_CACHE[key] = dict(
            T1=build_transform(cfg),
            E1=build_edge(cfg, 1, D_tbl, tile_off, CB),
            E2=build_edge(cfg, 2, D_tbl, tile_off, CB),
        )
    return _NC_CACHE[key]


def _prep_all(x, edge_index, W1, a_src1, a_dst1, W2, a_src2, a_dst2):
    cfg = make_cfg()
    G, GSZ, NV, SHPAD = cfg["G"], cfg["GSZ"], cfg["NV"], cfg["SHPAD"]
    N = cfg["N_RAW"]
    src, dst = edge_index[0], edge_index[1]
    D_tbl, blobs, orows, tile_off, CB = prep_graph(cfg, src, dst)

    orig = np.arange(N, dtype=np.int64)
    new_id = (orig % G) * GSZ + orig // G

    fT_all = np.zeros((cfg["F_IN"], NV), dtype=np.float16)
    fT_all[:, new_id] = np.asarray(x).T.astype(np.float16)

    W1p = _fold_w1(np.asarray(W1), np.asarray(a_src1), np.asarray(a_dst1),
                   cfg["HH"], cfg["CC"])
    W2p = _fold_w2(np.asarray(W2), np.asarray(a_src2), np.asarray(a_dst2))
    ident = np.eye(P, dtype=np.float16)
    return dict(cfg=cfg, D_tbl=D_tbl, blobs=blobs, orows=orows,
                tile_off=tile_off, CB=CB, new_id=new_id, fT_all=fT_all,
                W1p=W1p, W2p=W2p, ident=ident)


def _run_spmd(nc, in_maps, ncores):
    from concourse.bass_utils import run_bass_kernel_spmd
    res = run_bass_kernel_spmd(nc, in_maps, list(range(ncores)))
    return res.results


def _t1_inmaps(pp):
    cfg = pp["cfg"]
    SHPAD = cfg["SHPAD"]
    return [{"fT": np.ascontiguousarray(pp["fT_all"][:, r * SHPAD:(r + 1) * SHPAD]),
             "Wp": pp["W1p"]} for r in range(cfg["NCORES"])]


def _assemble_table1(pp, outs):
    cfg = pp["cfg"]
    NV, G, GSZ = cfg["NV"], cfg["G"], cfg["GSZ"]
    table1 = np.zeros((NV, 128), dtype=np.float16)
    table1[:, 0:96] = np.vstack([o["tshard"] for o in outs])
    C1 = table1[:, 72:80].astype(np.float32).max(axis=0) + 0.02
    for g in range(G):
        table1[g * GSZ + cfg["PAD_LOCAL"], 72:80] = NEG
    return table1, np.ascontiguousarray(C1, dtype=np.float32)


def _blob_with_own(pp, r, table):
    """Copy of core r's idx blob with each tile's own-row bytes filled in."""
    cfg = pp["cfg"]
    v = pp["orows"][r]
    m = v >= 0
    arr = np.zeros((cfg["SHPAD"], 128), np.float16)
    arr[m] = table[pp["new_id"][v[m]]]
    blob = pp["blobs"][r].copy()
    D_tbl, tile_off = pp["D_tbl"], pp["tile_off"]
    for t in range(cfg["NTILES"]):
        c = tile_off[t] + 8 * int(D_tbl[t].sum())
        blob[:, c:c + 128] = arr[t * P:(t + 1) * P].view(np.int16)
    return blob


def _e1_inmaps(pp, table1, C1, b1):
    cfg = pp["cfg"]
    return [{"table": table1, "idxb": _blob_with_own(pp, r, table1),
             "bias": np.ascontiguousarray(b1, dtype=np.float32),
             "cbnd": C1, "w2p": pp["W2p"], "ident": pp["ident"]}
            for r in range(cfg["NCORES"])]


def _assemble_table2(pp, outs):
    cfg = pp["cfg"]
    NV, G, GSZ = cfg["NV"], cfg["G"], cfg["GSZ"]
    table2 = np.zeros((NV, 128), dtype=np.float16)
    for r in range(cfg["NCORES"]):
        v = pp["orows"][r]
        m = v >= 0
        table2[pp["new_id"][v[m]], 0:48] = outs[r]["t2o"][m]
    C2 = np.array([table2[:, 41].astype(np.float32).max() + 0.02],
                  dtype=np.float32)
    for g in range(G):
        table2[g * GSZ + cfg["PAD_LOCAL"], 41] = NEG
    return table2, C2


def _e2_inmaps(pp, table2, C2, b2):
    cfg = pp["cfg"]
    return [{"table": table2, "idxb": _blob_with_own(pp, r, table2),
             "bias": np.ascontiguousarray(b2, dtype=np.float32),
             "cbnd": C2} for r in range(cfg["NCORES"])]


def _final_out(pp, outs):
    cfg = pp["cfg"]
    out = np.empty((cfg["N_RAW"], cfg["OUT_W"]), dtype=np.float32)
    for r in range(cfg["NCORES"]):
        v = pp["orows"][r]
        m = v >= 0
        out[v[m]] = outs[r]["out"][m]
    return out


def kernel(x, edge_index, W1, a_src1, a_dst1, b1, W2, a_src2, a_dst2, b2):
    x = np.asarray(x)
    edge_index = np.asarray(edge_index)
    pp = _prep_all(x, edge_index, W1, a_src1, a_dst1, W2, a_src2, a_dst2)
    cfg = pp["cfg"]
    progs = _get_programs(cfg, pp["D_tbl"], pp["tile_off"], pp["CB"])

    outs = _run_spmd(progs["T1"], _t1_inmaps(pp), cfg["NCORES"])
    table1, C1 = _assemble_table1(pp, outs)

    outs = _run_spmd(progs["E1"], _e1_inmaps(pp, table1, C1, np.asarray(b1)),
                     cfg["NCORES"])
    table2, C2 = _assemble_table2(pp, outs)

    outs = _run_spmd(progs["E2"], _e2_inmaps(pp, table2, C2, np.asarray(b2)),
                     cfg["NCORES"])
    return _final_out(pp, outs)


# --------------------------------------------------------------------------
# timing / profiling helpers (not used by the grader)
# --------------------------------------------------------------------------

def time_launches(inputs, repeats=2, hw=True):
    """Per-launch timing: TimelineSim (cost model) ns + optional HW walls.

    Returns {"T1": [sec...], "E1": [...], "E2": [...]} where values are
    sim exec times in seconds (so test.py's sum(min)*1e9 prints total ns).
    """
    import time as _time
    from concourse.timeline_sim import TimelineSim

    x = np.asarray(inputs["x"])
    ei = np.asarray(inputs["edge_index"])
    pp = _prep_all(x, ei, inputs["W1"], inputs["a_src1"], inputs["a_dst1"],
                   inputs["W2"], inputs["a_src2"], inputs["a_dst2"])
    cfg = pp["cfg"]
    progs = _get_programs(cfg, pp["D_tbl"], pp["tile_off"], pp["CB"])

    times = {}
    for name in ("T1", "E1", "E2"):
        tl = TimelineSim(progs[name], trace=False)
        ns = tl.simulate()
        times[name] = [ns / 1e9]
        print(f"  {name}: sim {ns:.0f} ns", flush=True)

    if hw:
        outs = _run_spmd(progs["T1"], _t1_inmaps(pp), cfg["NCORES"])
        table1, C1 = _assemble_table1(pp, outs)
        e1maps = _e1_inmaps(pp, table1, C1, np.asarray(inputs["b1"]))
        outs = _run_spmd(progs["E1"], e1maps, cfg["NCORES"])
        table2, C2 = _assemble_table2(pp, outs)
        e2maps = _e2_inmaps(pp, table2, C2, np.asarray(inputs["b2"]))
        for name, maps in (("T1", _t1_inmaps(pp)), ("E1", e1maps),
                           ("E2", e2maps)):
            walls = []
            for _ in range(repeats):
                t0 = _time.time()
                _run_spmd(progs[name], maps, cfg["NCORES"])
                walls.append(_time.time() - t0)
            print(f"  {name}: hw walls {[f'{w:.3f}' for w in walls]}", flush=True)
    return times


# revision 56
# speedup vs baseline: 1.0755x; 1.0088x over previous
"""Two-layer GAT on 8 Trainium2 NeuronCores (Bass/Tile, no collectives).

v2 design
---------
* 3 device launches, host does only data marshaling between them:
  - T1: data-parallel transform x@W1p -> table1 shard (each core 1/8 of
    nodes); 4 tiles batched per DMA (HWDGE-gen bound otherwise).
  - E1: edge pass layer 1 (gather + segment softmax + weighted sum) with
    the FULL table1 as a pre-staged input; fuses the layer-2 transform
    (elu(h)@W2p via PE transpose + matmul) so its output is directly the
    table2 shard.
  - E2: edge pass layer 2 -> final [N, 40] rows.
  Softmax bounds C (max a_src over nodes) come free on the host from the
  previous launch's output.
* Node ids relabeled into G=4 groups of GSZ rows (int16 gather indices,
  idx local to group). One pad row per group (a_src = -60000 => exp -> 0).
* Destination nodes are assigned to (core, tile) slots via a Morton-order
  sort of their per-group in-degree profiles: all 8 cores share the same
  per-tile slot counts D_tbl[t,g] (one program for all cores), and Morton
  clustering minimizes sum(max) padding (~1.38x vs 2.13x for degree sort).
  Each tile's own row arrives by direct DMA (extra slot), not gather, so
  no gather slots are reserved for self-loops / a_d extraction.
* Edge phase per 128-dst tile: dma_gather all incoming rows (<=8 slots =
  1024 descriptors per call -- the SWDGE ucode ring limit; 4 queues
  round-robin; 256B rows = [h | ones | a_s | a_d]), unnormalized-softmax
  with a per-(dst,head) upper bound m' = leaky(C + a_d) so exp stays in
  fp16, denominator via "ones" columns of the same multiply, message sum
  via in-place halving tree adds (packed fp16 -> 2x DVE mode), exp and
  ex-broadcast on the Act engine.
"""
import sys
sys.path.insert(0, "/opt/trn_rl_repo")

import numpy as np

P = 128
NEG = -60000.0

_NC_CACHE = {}


def _mybir():
    from concourse import mybir
    return mybir


def make_cfg(n_raw=100000, f_in=512, hh=8, cc=8, out_w=40, ncores=8, gsz=25088):
    g = 4
    sh_pad = 12544                       # 98 tiles * 128 rows per core
    nt = sh_pad // P
    assert ncores * sh_pad == g * gsz    # shard rows == table rows
    return dict(
        N_RAW=n_raw, F_IN=f_in, HH=hh, CC=cc, F_HID=hh * cc, OUT_W=out_w,
        NCORES=ncores, NTILES=nt, SHPAD=sh_pad,
        G=g, GSZ=gsz, PAD_LOCAL=gsz - 1, NV=g * gsz,
    )


# --------------------------------------------------------------------------
# host-side graph prep
# --------------------------------------------------------------------------

def _morton_key(c):
    m = np.zeros(len(c), dtype=np.int64)
    for b in range(6):
        for g in range(c.shape[1]):
            m |= ((c[:, g].astype(np.int64) >> b) & 1) << (b * c.shape[1] + g)
    return m


def prep_graph(cfg, src_orig, dst_orig):
    """Morton-window node placement + per-core gather-index blobs.

    Returns (D_tbl [NT,G], blobs [ncores x [128, CB] int16],
             orows [ncores x [SHPAD] original ids, -1 = empty], tile_off, CB)
    """
    G, GSZ = cfg["G"], cfg["GSZ"]
    NT, SHPAD, NC = cfg["NTILES"], cfg["SHPAD"], cfg["NCORES"]
    N = cfg["N_RAW"]
    PAD_LOCAL = cfg["PAD_LOCAL"]

    src = np.asarray(src_orig, dtype=np.int64)
    dst = np.asarray(dst_orig, dtype=np.int64)

    # Discrepancy-greedy coloring of SOURCE nodes into the G address groups:
    # balances every dst's in-neighborhood across groups, shrinking the
    # per-window max counts (gather padding) vs residue coloring.
    odeg = np.bincount(src, minlength=N)
    so = np.argsort(src, kind="stable")
    d_sorted = dst[so]
    sstart = np.searchsorted(src[so], np.arange(N + 1))
    cntm = np.zeros((N, G), np.int32)
    gsize = np.zeros(G, np.int64)
    group_of = np.empty(N, np.int8)
    CAP = GSZ - 1                      # rank GSZ-1 is the pad row
    for sn in np.argsort(-odeg, kind="stable"):
        a, b = sstart[sn], sstart[sn + 1]
        ds = d_sorted[a:b]
        tot = cntm[ds].sum(axis=0).astype(np.float64) if b > a else np.zeros(G)
        tot[gsize >= CAP] = 1e18
        g = int(np.argmin(tot + gsize * 1e-7))
        group_of[sn] = g
        gsize[g] += 1
        if b > a:
            cntm[ds, g] += 1
    rank_of = np.empty(N, dtype=np.int64)
    for g in range(G):
        nodes_g = np.nonzero(group_of == g)[0]
        rank_of[nodes_g] = np.arange(len(nodes_g))
    new_id = group_of.astype(np.int64) * GSZ + rank_of

    c = cntm                           # per-dst per-group edge counts
    order = np.argsort(_morton_key(c), kind="stable")
    W = NC * P
    ordp = np.full(NT * W, -1, dtype=np.int64)
    ordp[:N] = order
    owin = ordp.reshape(NT, NC, P)
    orows = [owin[:, r, :].ravel().copy() for r in range(NC)]

    core_of = np.full(N, -1, dtype=np.int64)
    pos_of = np.full(N, -1, dtype=np.int64)
    for r in range(NC):
        v = orows[r]
        m = v >= 0
        core_of[v[m]] = r
        pos_of[v[m]] = np.nonzero(m)[0]

    cnts = []
    per_core_fill = []
    for r in range(NC):
        em = core_of[dst] == r
        es = src[em]
        ep = pos_of[dst[em]]
        sg = group_of[es].astype(np.int64)
        sl = rank_of[es].astype(np.int16)
        key = ep * G + sg
        o = np.argsort(key, kind="stable")
        ks, vs = key[o], sl[o]
        cnt = np.bincount(ks, minlength=SHPAD * G).reshape(SHPAD, G)
        starts = np.zeros(SHPAD * G, dtype=np.int64)
        np.cumsum(cnt.ravel()[:-1], out=starts[1:])
        col = np.arange(len(ks)) - starts[ks]
        cnts.append(cnt)
        per_core_fill.append((ks, vs, col))

    cnt_all = np.stack(cnts)                          # [NC, SHPAD, G]
    D_tbl = cnt_all.reshape(NC, NT, P, G).max(axis=(0, 2))      # [NT, G]
    D_tbl = np.maximum(D_tbl, 1)
    dmax = int(D_tbl.max())

    tile_off = []
    off = 0
    for t in range(NT):
        tile_off.append(off)
        off += 8 * int(D_tbl[t].sum()) + 128   # +128: own-row bytes slot
    CB = off

    blobs = []
    for r in range(NC):
        ks, vs, col = per_core_fill[r]
        big = np.full((SHPAD, G, dmax), PAD_LOCAL, dtype=np.int16)
        big[ks // G, ks % G, col] = vs

        blob = np.empty((P, CB), dtype=np.int16)
        for t in range(NT):
            cpos = tile_off[t]
            for g in range(G):
                D = int(D_tbl[t, g])
                mat = big[t * P:(t + 1) * P, g, :D]      # [128, D]
                L = mat.T.ravel()                         # i = j*128 + p
                W16 = L.reshape(-1, 16).T                 # [16, 8*D]
                blob[:, cpos:cpos + 8 * D] = np.tile(W16, (8, 1))
                cpos += 8 * D
        blobs.append(blob)

    return D_tbl, blobs, orows, tile_off, CB, new_id


# --------------------------------------------------------------------------
# device programs
# --------------------------------------------------------------------------

def build_transform(cfg):
    """T1: fT shard [512, SHPAD] @ Wp [512, 96] -> tshard [SHPAD, 96] fp16
    (row = [h64 | ones8 | as8 | ad8 | 0*8]), plus column-max of as -> cmo."""
    import concourse.bass as bass
    import concourse.bacc as bacc
    import concourse.tile as tile
    mybir = _mybir()
    f16, f32 = mybir.dt.float16, mybir.dt.float32

    F_IN, SHPAD, NT = cfg["F_IN"], cfg["SHPAD"], cfg["NTILES"]
    RW = 96

    nc = bacc.Bacc("TRN2", target_bir_lowering=False, debug=False)
    fT = nc.dram_tensor("fT", [F_IN, SHPAD], f16, kind="ExternalInput")
    Wp = nc.dram_tensor("Wp", [F_IN, RW], f16, kind="ExternalInput")
    tshard = nc.dram_tensor("tshard", [SHPAD, RW], f16, kind="ExternalOutput")

    KCH = [(k, min(P, F_IN - k)) for k in range(0, F_IN, P)]

    with tile.TileContext(nc) as tc:
        import contextlib
        with contextlib.ExitStack() as ctx:
            singles = ctx.enter_context(tc.tile_pool(name="singles", bufs=1))
            xtp = ctx.enter_context(tc.tile_pool(name="xt", bufs=3))
            psp = ctx.enter_context(tc.tile_pool(name="ps", bufs=4, space="PSUM"))
            otp = ctx.enter_context(tc.tile_pool(name="ot", bufs=3))
            cp = ctx.enter_context(tc.tile_pool(name="cp", bufs=2))

            wts = []
            for kc, (k0, kn) in enumerate(KCH):
                wt = singles.tile([P, RW], f16, tag=f"w{kc}")
                nc.sync.dma_start(out=wt[:kn, :], in_=Wp[k0:k0 + kn, :])
                wts.append(wt)
            onesrow = singles.tile([P, RW], f32)
            nc.vector.memset(onesrow[:], 0.0)
            nc.vector.memset(onesrow[:, 64:72], 1.0)

            NKC = len(KCH)
            for t0 in range(0, NT, 4):
                nb = min(4, NT - t0)
                xt8 = xtp.tile([P, NKC, nb * P], f16, tag="x8")
                nc.sync.dma_start(
                    out=xt8[:],
                    in_=fT[:, t0 * P:(t0 + nb) * P]
                        .rearrange("(c k) n -> k c n", k=P))
                ot2 = otp.tile([P, nb, RW], f16, tag="ot2")
                for b in range(nb):
                    pt = psp.tile([P, RW], f32)
                    for kc in range(NKC):
                        nc.tensor.matmul(out=pt[:],
                                         lhsT=xt8[:, kc, b * P:(b + 1) * P],
                                         rhs=wts[kc][:],
                                         start=(kc == 0), stop=(kc == NKC - 1))
                    nc.vector.tensor_add(out=ot2[:, b], in0=pt[:], in1=onesrow[:])
                nc.sync.dma_start(
                    out=tshard[t0 * P:(t0 + nb) * P, :]
                        .rearrange("(b r) w -> r b w", r=P),
                    in_=ot2[:])

    nc.compile()
    return nc


def build_edge(cfg, layer, D_tbl, tile_off, CB):
    """Edge pass. layer 1: row [h64|ones8|as8|ad8], fused table2 emit.
    layer 2: row [g40|ones1|as1|ad1], final output rows."""
    import concourse.bass as bass
    import concourse.bacc as bacc
    import concourse.tile as tile
    mybir = _mybir()
    f16, f32, i16 = mybir.dt.float16, mybir.dt.float32, mybir.dt.int16

    G, GSZ, NV, NT = cfg["G"], cfg["GSZ"], cfg["NV"], cfg["NTILES"]
    SHPAD = cfg["SHPAD"]
    HH = cfg["HH"] if layer == 1 else 1
    CC = cfg["CC"] if layer == 1 else cfg["OUT_W"]
    CT = HH * CC
    MB = CT + HH                        # msg block incl ones cols
    AS_OFF, AD_OFF = MB, MB + HH
    OUT_W = CT
    fused = (layer == 1)
    W2RW = 48

    nc = bacc.Bacc("TRN2", target_bir_lowering=False, debug=False,
                   num_swdge_queues=4)
    table = nc.dram_tensor("table", [NV, 128], f16, kind="ExternalInput")
    idxb = nc.dram_tensor("idxb", [P, CB], i16, kind="ExternalInput")
    bias = nc.dram_tensor("bias", [OUT_W], f32, kind="ExternalInput")
    cbnd = nc.dram_tensor("cbnd", [HH], f32, kind="ExternalInput")
    if fused:
        w2p = nc.dram_tensor("w2p", [CT, W2RW], f16, kind="ExternalInput")
        ident = nc.dram_tensor("ident", [P, P], f16, kind="ExternalInput")
        t2o = nc.dram_tensor("t2o", [SHPAD, W2RW], f16, kind="ExternalOutput")
    else:
        outo = nc.dram_tensor("out", [SHPAD, OUT_W], f32, kind="ExternalOutput")

    qn = [0]

    def nextq():
        q = qn[0]
        qn[0] = (qn[0] + 1) % 4
        return q

    with tile.TileContext(nc) as tc:
        import contextlib
        with contextlib.ExitStack() as ctx:
            singles = ctx.enter_context(tc.tile_pool(name="singles", bufs=1))
            ip = ctx.enter_context(tc.tile_pool(name="ip", bufs=4))
            gp = ctx.enter_context(tc.tile_pool(name="gp", bufs=5))
            ep = ctx.enter_context(tc.tile_pool(name="ep", bufs=3))
            xp = ctx.enter_context(tc.tile_pool(name="xp", bufs=2))
            mp = ctx.enter_context(tc.tile_pool(name="mp", bufs=2))
            sp = ctx.enter_context(tc.tile_pool(name="sp", bufs=4))
            otp = ctx.enter_context(tc.tile_pool(name="otp", bufs=3))
            cp = ctx.enter_context(tc.tile_pool(name="cp", bufs=2))
            if fused:
                pstp = ctx.enter_context(
                    tc.tile_pool(name="pst", bufs=2, space="PSUM"))
                ps2p = ctx.enter_context(
                    tc.tile_pool(name="ps2", bufs=2, space="PSUM"))

            # ---- singles ----
            bias_ap = bias[:]
            bias_b = bass.AP(tensor=bias_ap.tensor, offset=bias_ap.offset,
                             ap=[[0, P]] + list(bias_ap.ap))
            bt = singles.tile([P, OUT_W], f32)
            nc.sync.dma_start(out=bt[:], in_=bias_b)
            cb_ap = cbnd[:]
            cb_b = bass.AP(tensor=cb_ap.tensor, offset=cb_ap.offset,
                           ap=[[0, P]] + list(cb_ap.ap))
            cbf = singles.tile([P, HH], f32)
            nc.sync.dma_start(out=cbf[:], in_=cb_b)
            cb16 = singles.tile([P, HH], f16)
            nc.vector.tensor_copy(out=cb16[:], in_=cbf[:])
            if fused:
                w2t = singles.tile([CT, W2RW], f16)
                nc.sync.dma_start(out=w2t[:], in_=w2p[:, :])
                idt = singles.tile([P, P], f16)
                nc.sync.dma_start(out=idt[:], in_=ident[:, :])
                ones2 = singles.tile([P, W2RW], f32)
                nc.vector.memset(ones2[:], 0.0)
                nc.vector.memset(ones2[:, 40:41], 1.0)

            for t in range(NT):
                Ds = [int(D_tbl[t, g]) for g in range(G)]
                SD = sum(Ds)
                SDP = SD + 1      # + own-row slot (direct DMA, no gather)
                it = ip.tile([P, 8 * SD + 128], i16, tag="idx")
                nc.sync.dma_start(
                    out=it[:],
                    in_=idxb[:, tile_off[t]:tile_off[t] + 8 * SD + 128])

                Gt = gp.tile([P, SDP, 128], f16, tag="G")
                c0 = 0
                ic = 0
                GCH = 8           # 1024 descriptors per call (ucode ring limit)
                for g in range(G):
                    D = Ds[g]
                    for d0 in range(0, D, GCH):
                        dn = min(GCH, D - d0)
                        nc.gpsimd.dma_gather(
                            out_ap=Gt[:, c0:c0 + dn, :],
                            in_ap=table[g * GSZ:, :],
                            idxs_ap=it[:, ic:ic + 8 * dn],
                            num_idxs=P * dn,
                            num_idxs_reg=P * dn,
                            elem_size=128,
                            queue_num=nextq(),
                        )
                        c0 += dn
                        ic += 8 * dn
                nc.vector.tensor_copy(
                    out=Gt[:, SD, :],
                    in_=it[:, 8 * SD:8 * SD + 128].bitcast(f16))

                # ad_own [p, H] straight from the own-row slot
                adt = sp.tile([P, HH], f16, tag="ad")
                nc.vector.tensor_copy(out=adt[:],
                                      in_=Gt[:, SD, AD_OFF:AD_OFF + HH])

                # nm = -leaky(C + ad_own)   [p, H] f16
                nm = sp.tile([P, HH], f16, tag="nm")
                nc.vector.tensor_add(out=nm[:], in0=adt[:], in1=cb16[:])
                nc.vector.scalar_tensor_tensor(
                    out=nm[:], in0=nm[:], scalar=0.2, in1=nm[:],
                    op0=mybir.AluOpType.mult, op1=mybir.AluOpType.max)
                nc.vector.tensor_scalar_mul(out=nm[:], in0=nm[:], scalar1=-1.0)

                # z = as + ad_own ; L = leaky(z) ; y = L - m' ; ex = exp(y)
                zt = ep.tile([P, SDP, HH], f16, tag="z")
                nc.vector.tensor_tensor(
                    out=zt[:], in0=Gt[:, :, AS_OFF:AS_OFF + HH],
                    in1=adt[:].unsqueeze(1).broadcast_to([P, SDP, HH]),
                    op=mybir.AluOpType.add)
                zf = zt[:].rearrange("p d h -> p (d h)")
                nc.vector.scalar_tensor_tensor(
                    out=zf, in0=zf, scalar=0.2, in1=zf,
                    op0=mybir.AluOpType.mult, op1=mybir.AluOpType.max)
                nc.vector.tensor_tensor(
                    out=zt[:], in0=zt[:],
                    in1=nm[:].unsqueeze(1).broadcast_to([P, SDP, HH]),
                    op=mybir.AluOpType.add)
                ext = ep.tile([P, SDP, HH], f16, tag="ex")
                nc.scalar.activation(
                    out=ext[:].rearrange("p d h -> p (d h)"), in_=zf,
                    func=mybir.ActivationFunctionType.Exp)

                # expand ex -> [p, d, MB] (Act engine)
                exm = xp.tile([P, SDP, MB], f16, tag="exm")
                nc.scalar.activation(
                    out=exm[:, :, 0:CT].rearrange("p d (h c) -> p d h c", h=HH),
                    in_=ext[:].unsqueeze(3).broadcast_to([P, SDP, HH, CC]),
                    func=mybir.ActivationFunctionType.Copy)
                nc.scalar.activation(
                    out=exm[:, :, CT:MB], in_=ext[:],
                    func=mybir.ActivationFunctionType.Copy)

                # msg = feat * ex ; tree-reduce over slots
                mg = mp.tile([P, SDP, MB], f16, tag="mg")
                nc.vector.tensor_tensor(out=mg[:], in0=Gt[:, :, 0:MB],
                                        in1=exm[:], op=mybir.AluOpType.mult)
                cur = SDP
                while cur > 2:
                    half = cur // 2
                    nc.vector.tensor_add(out=mg[:, 0:half],
                                         in0=mg[:, 0:half],
                                         in1=mg[:, cur - half:cur])
                    cur = cur - half
                redf = sp.tile([P, MB], f32, tag="red")
                if cur == 2:
                    nc.vector.tensor_add(out=redf[:], in0=mg[:, 0], in1=mg[:, 1])
                else:
                    nc.vector.tensor_copy(out=redf[:], in_=mg[:, 0])

                # normalize + bias
                rd = sp.tile([P, HH], f32, tag="rd")
                nc.vector.tensor_scalar_add(out=rd[:], in0=redf[:, CT:MB],
                                            scalar1=1e-16)
                nc.vector.reciprocal(out=rd[:], in_=rd[:])
                o1 = sp.tile([P, CT], f32, tag="o1")
                nc.vector.tensor_tensor(
                    out=o1[:].rearrange("p (h c) -> p h c", h=HH),
                    in0=redf[:, 0:CT].rearrange("p (h c) -> p h c", h=HH),
                    in1=rd[:].unsqueeze(2).broadcast_to([P, HH, CC]),
                    op=mybir.AluOpType.mult)
                nc.vector.tensor_add(out=o1[:], in0=o1[:], in1=bt[:])

                if fused:
                    # elu -> fp16
                    t1 = sp.tile([P, CT], f32, tag="t1")
                    nc.vector.tensor_scalar_min(out=t1[:], in0=o1[:], scalar1=0.0)
                    nc.scalar.activation(out=t1[:], in_=t1[:],
                                         func=mybir.ActivationFunctionType.Exp)
                    t2e = sp.tile([P, CT], f32, tag="t2e")
                    nc.vector.tensor_scalar_max(out=t2e[:], in0=o1[:], scalar1=0.0)
                    nc.vector.tensor_add(out=t1[:], in0=t1[:], in1=t2e[:])
                    h16 = sp.tile([P, CT], f16, tag="h16")
                    nc.vector.tensor_scalar_add(out=h16[:], in0=t1[:], scalar1=-1.0)
                    # transpose h16 -> [CT, 128] then @ W2p -> table2 rows
                    psT = pstp.tile([CT, P], f16)
                    nc.tensor.transpose(out=psT[:], in_=h16[:], identity=idt[:])
                    hT = otp.tile([CT, P], f16, tag="hT")
                    nc.vector.tensor_copy(out=hT[:], in_=psT[:])
                    ps2 = ps2p.tile([P, W2RW], f32)
                    nc.tensor.matmul(out=ps2[:], lhsT=hT[:], rhs=w2t[:],
                                     start=True, stop=True)
                    t2t = otp.tile([P, W2RW], f16, tag="t2t")
                    nc.vector.tensor_add(out=t2t[:], in0=ps2[:], in1=ones2[:])
                    nc.sync.dma_start(out=t2o[t * P:(t + 1) * P, :], in_=t2t[:])
                else:
                    nc.sync.dma_start(out=outo[t * P:(t + 1) * P, :], in_=o1[:])

    nc.compile()
    return nc


# --------------------------------------------------------------------------
# host orchestration
# --------------------------------------------------------------------------

def _fold_w1(W1, a_src, a_dst, hh, cc):
    W1r = W1.reshape(W1.shape[0], hh, cc)
    ws = np.einsum("khc,hc->kh", W1r, a_src)
    wd = np.einsum("khc,hc->kh", W1r, a_dst)
    z8 = np.zeros((W1.shape[0], 8), np.float32)
    return np.concatenate([W1, z8, ws, wd, z8], axis=1).astype(np.float16)


def _fold_w2(W2, a_src2, a_dst2):
    z1 = np.zeros((W2.shape[0], 1), np.float32)
    z5 = np.zeros((W2.shape[0], 5), np.float32)
    return np.concatenate([W2, z1, (W2 @ a_src2[0])[:, None],
                           (W2 @ a_dst2[0])[:, None], z5], axis=1).astype(np.float16)


def _get_programs(cfg, D_tbl, tile_off, CB):
    key = (CB, D_tbl.tobytes())
    if key not in _NC_CACHE:
        _NC_CACHE[key] = dict(
            T1=build_transform(cfg),
            E1=build_edge(cfg, 1, D_tbl, tile_off, CB),
            E2=build_edge(cfg, 2, D_tbl, tile_off, CB),
        )
    return _NC_CACHE[key]


def _prep_all(x, edge_index, W1, a_src1, a_dst1, W2, a_src2, a_dst2):
    cfg = make_cfg()
    G, GSZ, NV, SHPAD = cfg["G"], cfg["GSZ"], cfg["NV"], cfg["SHPAD"]
    N = cfg["N_RAW"]
    src, dst = edge_index[0], edge_index[1]
    D_tbl, blobs, orows, tile_off, CB, new_id = prep_graph(cfg, src, dst)

    fT_all = np.zeros((cfg["F_IN"], NV), dtype=np.float16)
    fT_all[:, new_id] = np.asarray(x).T.astype(np.float16)

    W1p = _fold_w1(np.asarray(W1), np.asarray(a_src1), np.asarray(a_dst1),
                   cfg["HH"], cfg["CC"])
    W2p = _fold_w2(np.asarray(W2), np.asarray(a_src2), np.asarray(a_dst2))
    ident = np.eye(P, dtype=np.float16)
    return dict(cfg=cfg, D_tbl=D_tbl, blobs=blobs, orows=orows,
                tile_off=tile_off, CB=CB, new_id=new_id, fT_all=fT_all,
                W1p=W1p, W2p=W2p, ident=ident)


def _run_spmd(nc, in_maps, ncores):
    from concourse.bass_utils import run_bass_kernel_spmd
    res = run_bass_kernel_spmd(nc, in_maps, list(range(ncores)))
    return res.results


def _t1_inmaps(pp):
    cfg = pp["cfg"]
    SHPAD = cfg["SHPAD"]
    return [{"fT": np.ascontiguousarray(pp["fT_all"][:, r * SHPAD:(r + 1) * SHPAD]),
             "Wp": pp["W1p"]} for r in range(cfg["NCORES"])]


def _assemble_table1(pp, outs):
    cfg = pp["cfg"]
    NV, G, GSZ = cfg["NV"], cfg["G"], cfg["GSZ"]
    table1 = np.zeros((NV, 128), dtype=np.float16)
    table1[:, 0:96] = np.vstack([o["tshard"] for o in outs])
    C1 = table1[:, 72:80].astype(np.float32).max(axis=0) + 0.02
    for g in range(G):
        table1[g * GSZ + cfg["PAD_LOCAL"], 72:80] = NEG
    return table1, np.ascontiguousarray(C1, dtype=np.float32)


def _blob_with_own(pp, r, table):
    """Copy of core r's idx blob with each tile's own-row bytes filled in."""
    cfg = pp["cfg"]
    v = pp["orows"][r]
    m = v >= 0
    arr = np.zeros((cfg["SHPAD"], 128), np.float16)
    arr[m] = table[pp["new_id"][v[m]]]
    blob = pp["blobs"][r].copy()
    D_tbl, tile_off = pp["D_tbl"], pp["tile_off"]
    for t in range(cfg["NTILES"]):
        c = tile_off[t] + 8 * int(D_tbl[t].sum())
        blob[:, c:c + 128] = arr[t * P:(t + 1) * P].view(np.int16)
    return blob


def _e1_inmaps(pp, table1, C1, b1):
    cfg = pp["cfg"]
    return [{"table": table1, "idxb": _blob_with_own(pp, r, table1),
             "bias": np.ascontiguousarray(b1, dtype=np.float32),
             "cbnd": C1, "w2p": pp["W2p"], "ident": pp["ident"]}
            for r in range(cfg["NCORES"])]


def _assemble_table2(pp, outs):
    cfg = pp["cfg"]
    NV, G, GSZ = cfg["NV"], cfg["G"], cfg["GSZ"]
    table2 = np.zeros((NV, 128), dtype=np.float16)
    for r in range(cfg["NCORES"]):
        v = pp["orows"][r]
        m = v >= 0
        table2[pp["new_id"][v[m]], 0:48] = outs[r]["t2o"][m]
    C2 = np.array([table2[:, 41].astype(np.float32).max() + 0.02],
                  dtype=np.float32)
    for g in range(G):
        table2[g * GSZ + cfg["PAD_LOCAL"], 41] = NEG
    return table2, C2


def _e2_inmaps(pp, table2, C2, b2):
    cfg = pp["cfg"]
    return [{"table": table2, "idxb": _blob_with_own(pp, r, table2),
             "bias": np.ascontiguousarray(b2, dtype=np.float32),
             "cbnd": C2} for r in range(cfg["NCORES"])]


def _final_out(pp, outs):
    cfg = pp["cfg"]
    out = np.empty((cfg["N_RAW"], cfg["OUT_W"]), dtype=np.float32)
    for r in range(cfg["NCORES"]):
        v = pp["orows"][r]
        m = v >= 0
        out[v[m]] = outs[r]["out"][m]
    return out


def kernel(x, edge_index, W1, a_src1, a_dst1, b1, W2, a_src2, a_dst2, b2):
    x = np.asarray(x)
    edge_index = np.asarray(edge_index)
    pp = _prep_all(x, edge_index, W1, a_src1, a_dst1, W2, a_src2, a_dst2)
    cfg = pp["cfg"]
    progs = _get_programs(cfg, pp["D_tbl"], pp["tile_off"], pp["CB"])

    outs = _run_spmd(progs["T1"], _t1_inmaps(pp), cfg["NCORES"])
    table1, C1 = _assemble_table1(pp, outs)

    outs = _run_spmd(progs["E1"], _e1_inmaps(pp, table1, C1, np.asarray(b1)),
                     cfg["NCORES"])
    table2, C2 = _assemble_table2(pp, outs)

    outs = _run_spmd(progs["E2"], _e2_inmaps(pp, table2, C2, np.asarray(b2)),
                     cfg["NCORES"])
    return _final_out(pp, outs)


# --------------------------------------------------------------------------
# timing / profiling helpers (not used by the grader)
# --------------------------------------------------------------------------

def time_launches(inputs, repeats=2, hw=True):
    """Per-launch timing: TimelineSim (cost model) ns + optional HW walls.

    Returns {"T1": [sec...], "E1": [...], "E2": [...]} where values are
    sim exec times in seconds (so test.py's sum(min)*1e9 prints total ns).
    """
    import time as _time
    from concourse.timeline_sim import TimelineSim

    x = np.asarray(inputs["x"])
    ei = np.asarray(inputs["edge_index"])
    pp = _prep_all(x, ei, inputs["W1"], inputs["a_src1"], inputs["a_dst1"],
                   inputs["W2"], inputs["a_src2"], inputs["a_dst2"])
    cfg = pp["cfg"]
    progs = _get_programs(cfg, pp["D_tbl"], pp["tile_off"], pp["CB"])

    times = {}
    for name in ("T1", "E1", "E2"):
        tl = TimelineSim(progs[name], trace=False)
        ns = tl.simulate()
        times[name] = [ns / 1e9]
        print(f"  {name}: sim {ns:.0f} ns", flush=True)

    if hw:
        outs = _run_spmd(progs["T1"], _t1_inmaps(pp), cfg["NCORES"])
        table1, C1 = _assemble_table1(pp, outs)
        e1maps = _e1_inmaps(pp, table1, C1, np.asarray(inputs["b1"]))
        outs = _run_spmd(progs["E1"], e1maps, cfg["NCORES"])
        table2, C2 = _assemble_table2(pp, outs)
        e2maps = _e2_inmaps(pp, table2, C2, np.asarray(inputs["b2"]))
        for name, maps in (("T1", _t1_inmaps(pp)), ("E1", e1maps),
                           ("E2", e2maps)):
            walls = []
            for _ in range(repeats):
                t0 = _time.time()
                _run_spmd(progs[name], maps, cfg["NCORES"])
                walls.append(_time.time() - t0)
            print(f"  {name}: hw walls {[f'{w:.3f}' for w in walls]}", flush=True)
    return times
